# revision 30
# baseline (speedup 1.0000x reference)
"""Trainium2 Bass kernel for a GNN message-passing block (3x3 unfold +
1x1 convs), sharded over 8 NeuronCores along the W (azimuth) axis.

Layout strategy ("pixel-split planes"):
  Every on-chip tensor is [128, L] where partitions 0-63 hold the 64
  channels for the first half of this core's rows (half A) and
  partitions 64-127 hold the same channels for the second half (half B).
  All 1x1 convs become matmuls with block-diagonal [128,128] weights so
  one instruction serves both halves; all elementwise ops run at the
  full 128-lane width.

Algebraic restructuring vs the reference:
  - BN folded into conv weights/bias on the host.
  - pos_w1 @ rel_k == shift_k(u) - u with u = pos_w1 @ cart (computed
    once instead of 9x).
  - center tap (k=4): pe1 = relu(pb1) is constant, so its fusion
    contribution folds into the weights: F1_4' = F1_4 * c4.
  - x >= 0 (relu output), so relu ordering around the pe2*x product is
    flexible.

Halo handling: the host passes each core its W-slice plus one halo
column per side (zeros at the global edges). The only values the kernel
cannot reproduce are the global-edge output columns (x at a padded
column is not zero after the biased projection), so the host recomputes
output columns 0 and W-1 in numpy and overwrites them.
"""

import numpy as np
import ml_dtypes

import concourse.bass as bass
import concourse.bacc as bacc
import concourse.mybir as mybir
import concourse.tile as tile
from concourse import bass_utils

FP32 = mybir.dt.float32
FP32R = mybir.dt.float32r
BF16 = mybir.dt.bfloat16
Relu = mybir.ActivationFunctionType.Relu
Alu = mybir.AluOpType

EPS = 1e-5
B, CIN, COUT, H, W = 1, 64, 64, 64, 2048
NCORES = 8
WL = W // NCORES          # 256 interior columns per core
WP = WL + 2               # 258 columns incl. 1-col halo each side
HH = H // 2               # 32 output rows per half
ROWS = HH + 2             # 34 plane rows (1 halo/pad row each side)
L = ROWS * WP             # 8772 flat plane length (image coords)
DOFF = 2                  # plane data offset (guard elements)
LP = L + 4                # padded plane length
INT0 = DOFF + WP          # first interior element (row 1, col 0)
LINT = HH * WP            # 8256 interior length
CHUNK_A = 1024            # stage-A chunk
CHUNK_B = 1024            # stage-B chunk

# neighbor taps (di, dj), excluding the center (0,0)
TAPS = [(di, dj) for di in (-1, 0, 1) for dj in (-1, 0, 1) if not (di == 0 and dj == 0)]

# staged output-store quarters (flushed as soon as their chunks complete)
OUT_FLUSH = [(0, 2048), (2048, 4096), (4096, 6144), (6144, LINT)]


def _fold(w, bn):
    g, b, m, v = [np.asarray(t, np.float64) for t in bn]
    s = g / np.sqrt(v + EPS)
    return (np.asarray(w, np.float64) * s[:, None]).astype(np.float32), (
        b - m * s
    ).astype(np.float32)


def _bd(w):
    """[O, C] conv weight -> [128, 128] block-diag lhsT ([K, M] layout)."""
    o, c = w.shape
    out = np.zeros((128, 128), np.float32)
    out[0:c, 0:o] = w.T
    out[64 : 64 + c, 64 : 64 + o] = w.T
    return out


def _prep_weights(params):
    p = params
    w1, b1 = _fold(p["proj_w1"], p["proj_bn1"])
    w2, b2 = _fold(p["proj_w2"], p["proj_bn2"])
    ws, bs = _fold(p["proj_ws"], p["proj_bns"])
    p1, pb1 = _fold(p["pos_w1"], p["pos_bn1"])
    p2, pb2 = _fold(p["pos_w2"], p["pos_bn2"])
    f1, fb1 = _fold(p["fus_w1"], p["fus_bn1"])
    f2, fb2 = _fold(p["fus_w2"], p["fus_bn2"])
    bx = (b2 + bs).astype(np.float32)

    # center tap constant: pe2_center = relu(p2 @ relu(pb1) + pb2)
    c4 = np.maximum(p2 @ np.maximum(pb1, 0.0) + pb2, 0.0).astype(np.float32)
    f1k = [f1[:, k::9].copy() for k in range(9)]
    f1k[4] = f1k[4] * c4[None, :]

    # wb: bf16 weights packed side by side [128, 15*128]:
    #   0=w1 1=w2 2=ws 3=p2 4..12=f1(k) 13=f2 14=p1
    mats = [w1, w2, ws, p2] + f1k + [f2, p1]
    wb = np.concatenate([_bd(m) for m in mats], axis=1).astype(ml_dtypes.bfloat16)
    biases = np.stack(
        [np.concatenate([b, b]) for b in (b1, bx, pb1, pb2, fb1, fb2)], axis=1
    ).astype(np.float32)  # [128, 6]
    return wb, biases


def _plane_pair(img, j0):
    """img [C, H, W] (numpy f32) -> [2*C?, ...] no: [C-part pair planes].

    Returns [128-ish rows, L] with rows 0:C = half-A plane, rows
    64:64+C = half-B plane, flattened [ROWS, WP] per half. Halo columns
    come from the global image with zero padding at the W edges; halo
    rows (A row 0, B row 33) are zero."""
    ch = img.shape[0]
    wide = np.zeros((ch, H, WP), np.float32)
    lo = max(j0 - 1, 0)
    hi = min(j0 + WL + 1, W)
    wide[:, :, lo - (j0 - 1) : hi - (j0 - 1)] = img[:, :, lo:hi]
    out = np.zeros((128, L), np.float32)
    a = np.zeros((ch, ROWS, WP), np.float32)
    a[:, 1:34, :] = wide[:, 0:33, :]
    b = np.zeros((ch, ROWS, WP), np.float32)
    b[:, 0:33, :] = wide[:, 31:64, :]
    out[0:ch] = a.reshape(ch, L)
    out[64 : 64 + ch] = b.reshape(ch, L)
    return out


def build_kernel(tc, repeat=1):
    """Emit the per-core kernel IR. DRAM tensors are declared here.

    repeat > 1 re-emits the whole computation (timing aid: per-iteration
    time = wall-clock slope between two repeat counts)."""
    nc = tc.nc
    fio = nc.dram_tensor("fio", [128, L], BF16, kind="ExternalInput").ap()
    cart6 = nc.dram_tensor("cart6", [6, L], BF16, kind="ExternalInput").ap()
    wb_d = nc.dram_tensor("wb", [128, 15 * 128], BF16, kind="ExternalInput").ap()
    bias_d = nc.dram_tensor("bias", [128, 6], FP32, kind="ExternalInput").ap()
    out_d = nc.dram_tensor("out", [128, LINT], FP32, kind="ExternalOutput").ap()

    from contextlib import ExitStack

    with ExitStack() as ctx:
        const = ctx.enter_context(tc.tile_pool(name="const", bufs=1))
        planes = ctx.enter_context(tc.tile_pool(name="planes", bufs=1))

        wb_t = const.tile([128, 15 * 128], BF16)
        nc.sync.dma_start(wb_t[:], wb_d)
        bias_t = const.tile([128, 6], FP32)
        nc.sync.dma_start(bias_t[:], bias_d)

        def WB(i):
            return wb_t[:, i * 128 : (i + 1) * 128]

        def BIAS(i):
            return bias_t[:, i : i + 1]

        u_pl = planes.tile([128, LP], BF16)
        u1_pl = planes.tile([128, LP], BF16)
        x_pl = planes.tile([128, LP], BF16)
        x1_pl = planes.tile([128, LP], BF16)
        f_pl = planes.tile([128, L], BF16)
        c_pl = planes.tile([128, L], BF16)
        o_pl = planes.tile([128, LINT], FP32)

        for _rep in range(repeat):
            _build_iteration(
                tc, ctx, fio, cart6, bias_t, WB, BIAS,
                u_pl, u1_pl, x_pl, x1_pl, f_pl, c_pl, o_pl, out_d,
            )


def _build_iteration(
    tc, ctx, fio, cart6, bias_t, WB, BIAS,
    u_pl, u1_pl, x_pl, x1_pl, f_pl, c_pl, o_pl, out_d,
):
    nc = tc.nc
    if True:
        # f input: few big DMAs (descriptor-count bound on the issuing seq)
        NQ = 4
        qs = [L * i // NQ for i in range(NQ + 1)]
        for i in range(NQ):
            nc.sync.dma_start(f_pl[:, qs[i] : qs[i + 1]], fio[:, qs[i] : qs[i + 1]])
        # cart plane: zero once, then land the 3 channels per half
        nc.gpsimd.memset(c_pl[:], 0.0)
        nc.sync.dma_start(c_pl[0:3, :], cart6[0:3, :])
        nc.sync.dma_start(c_pl[64:67, :], cart6[3:6, :])

        # ---- stage A: u = p1@cart ; x = relu(w2@relu(w1@f+b1) + ws@f + bx)
        # evac engine split: u + h1 on DVE (idle in stage A), x on ACT.
        with (
            tc.tile_pool(name="aio", bufs=3) as aio,
            tc.tile_pool(name="astg", bufs=3) as astg,
            tc.tile_pool(name="apsum", bufs=1, space="PSUM") as apsum,
        ):
            for c0 in range(0, L, CHUNK_A):
                n = min(CHUNK_A, L - c0)
                d = DOFF + c0
                slices = [(s, min(512, n - s)) for s in range(0, n, 512)]
                pu = apsum.tile([128, CHUNK_A], FP32, tag="pu")
                for s, sn in slices:
                    nc.tensor.matmul(
                        pu[:, s : s + sn], WB(14), c_pl[:, c0 + s : c0 + s + sn],
                        start=True, stop=True,
                    )
                nc.scalar.copy(u_pl[:, d : d + n], pu[:, 0:n])

                ph = apsum.tile([128, CHUNK_A], FP32, tag="ph")
                for s, sn in slices:
                    nc.tensor.matmul(
                        ph[:, s : s + sn], WB(0), f_pl[:, c0 + s : c0 + s + sn],
                        start=True, stop=True,
                    )
                h1_t = astg.tile([128, CHUNK_A], BF16, tag="h1")
                nc.vector.tensor_scalar(
                    h1_t[:, 0:n], ph[:, 0:n], BIAS(0), 0.0, Alu.add, Alu.max
                )

                px = apsum.tile([128, CHUNK_A], FP32, tag="px")
                for s, sn in slices:
                    nc.tensor.matmul(
                        px[:, s : s + sn], WB(1), h1_t[:, s : s + sn],
                        start=True, stop=False,
                    )
                    nc.tensor.matmul(
                        px[:, s : s + sn], WB(2), f_pl[:, c0 + s : c0 + s + sn],
                        start=False, stop=True,
                    )
                nc.scalar.activation(x_pl[:, d : d + n], px[:, 0:n], Relu, bias=BIAS(1))

        # guard elements + pad rows, then the +1-shifted copies
        for pl in (u_pl, x_pl):
            nc.vector.memset(pl[:, 0:DOFF], 0.0)
            nc.vector.memset(pl[:, LP - 2 : LP], 0.0)
        # zero x at the out-of-image pad rows (A: image row -1, B: row H)
        nc.vector.memset(x_pl[0:64, DOFF : DOFF + WP], 0.0)
        nc.vector.memset(x_pl[64:128, DOFF + L - WP : DOFF + L], 0.0)
        # shifted planes: s1[m] = s[m-1], split so stage B can start on
        # early quarters before stage A fully finishes
        NSPLIT = 4
        bnds = [1 + (LP - 1) * i // NSPLIT for i in range(NSPLIT + 1)]
        for s1_pl, s_pl in ((u1_pl, u_pl), (x1_pl, x_pl)):
            for bi in range(NSPLIT):
                lo, hi = bnds[bi], bnds[bi + 1]
                nc.vector.tensor_copy(s1_pl[:, lo:hi], s_pl[:, lo - 1 : hi - 1])
            nc.vector.memset(s1_pl[:, 0:1], 0.0)

        # ---- stage B
        def shifted(base, s1, c, n, di, dj):
            off = c + di * WP
            if dj == -1:
                return s1[:, off : off + n]
            if dj == 0:
                return base[:, off : off + n]
            return s1[:, off + 2 : off + 2 + n]

        with (
            tc.tile_pool(name="bstg", bufs=4) as bstg,
            tc.tile_pool(name="gpool", bufs=2) as gpool,
            tc.tile_pool(name="bpsum", bufs=3, space="PSUM") as bpsum,
            tc.tile_pool(name="bpsum1", bufs=1, space="PSUM") as bpsum1,
        ):
            for c0 in range(0, LINT, CHUNK_B):
                n = min(CHUNK_B, LINT - c0)
                c = INT0 + c0
                slices = [(s, min(512, n - s)) for s in range(0, n, 512)]

                # pe1/pe2/g for the 8 non-center taps
                gs = []
                for ki, (di, dj) in enumerate(TAPS):
                    pe1 = bstg.tile([128, CHUNK_B], BF16, tag="pe1")
                    nc.vector.tensor_sub(
                        pe1[:, 0:n], shifted(u_pl, u1_pl, c, n, di, dj), u_pl[:, c : c + n]
                    )
                    eng = nc.gpsimd if ki in (1, 4, 6) else nc.vector
                    eng.tensor_scalar(
                        pe1[:, 0:n], pe1[:, 0:n], BIAS(2), 0.0, Alu.add, Alu.max
                    )
                    pp = bpsum.tile([128, CHUNK_B], FP32, tag="pe2")
                    for s, sn in slices:
                        nc.tensor.matmul(
                            pp[:, s : s + sn], WB(3), pe1[:, s : s + sn],
                            start=True, stop=True,
                        )
                    g = gpool.tile([128, CHUNK_B], BF16, tag=f"g{ki}")
                    nc.scalar.activation(g[:, 0:n], pp[:, 0:n], Relu, bias=BIAS(3))
                    nc.vector.tensor_mul(
                        g[:, 0:n], g[:, 0:n], shifted(x_pl, x1_pl, c, n, di, dj)
                    )
                    gs.append(g)

                # fusion accumulation: center (folded) + 8 taps
                acc = bpsum1.tile([128, CHUNK_B], FP32, tag="acc")
                for s, sn in slices:
                    nc.tensor.matmul(
                        acc[:, s : s + sn], WB(4 + 4), x_pl[:, c + s : c + s + sn],
                        start=True, stop=False,
                    )
                for ki, (di, dj) in enumerate(TAPS):
                    k = (di + 1) * 3 + (dj + 1)
                    for si, (s, sn) in enumerate(slices):
                        nc.tensor.matmul(
                            acc[:, s : s + sn], WB(4 + k), gs[ki][:, s : s + sn],
                            start=False, stop=(ki == len(TAPS) - 1),
                        )
                geo = bstg.tile([128, CHUNK_B], BF16, tag="geo")
                nc.scalar.activation(geo[:, 0:n], acc[:, 0:n], Relu, bias=BIAS(4))

                po = bpsum.tile([128, CHUNK_B], FP32, tag="pe2")
                for s, sn in slices:
                    nc.tensor.matmul(
                        po[:, s : s + sn], WB(13), geo[:, s : s + sn],
                        start=True, stop=True,
                    )
                nc.scalar.activation(o_pl[:, c0 : c0 + n], po[:, 0:n], Relu, bias=BIAS(5))
                # staged output DMA: flush completed quarters so the store
                # overlaps compute instead of serializing at the end
                done = c0 + n
                prev = c0
                for q0, q1 in OUT_FLUSH:
                    if prev < q1 <= done:
                        nc.sync.dma_start(out_d[:, q0:q1], o_pl[:, q0:q1])


_CACHE = {}


def _get_compiled(repeat=1):
    key = ("nc", repeat)
    if key not in _CACHE:
        nc = bacc.Bacc("TRN2", target_bir_lowering=False, debug=False)
        with tile.TileContext(nc) as tc:
            build_kernel(tc, repeat=repeat)
        nc.compile()
        _CACHE[key] = nc
    return _CACHE[key]


def _prep_in_maps(features, cart, params):
    features = np.asarray(features, np.float32)
    cart = np.asarray(cart, np.float32)
    wb, biases = _prep_weights(params)
    in_maps = []
    for i in range(NCORES):
        j0 = i * WL
        fio = _plane_pair(features[0], j0).astype(ml_dtypes.bfloat16)
        cp = _plane_pair(cart[0], j0)
        cart6 = (
            np.concatenate([cp[0:3], cp[64:67]], axis=0).astype(ml_dtypes.bfloat16)
        )
        in_maps.append({"fio": fio, "cart6": cart6, "wb": wb, "bias": biases})
    return in_maps


def _np_reference(features, cart, params):
    """Pure-numpy mirror of reference.reference (same zero-pad unfold)."""
    p = params

    def bn(x, g, b, m, v):
        sh = (1, -1) + (1,) * (x.ndim - 2)
        s = g / np.sqrt(v + EPS)
        return x * s.reshape(sh) + (b - m * s).reshape(sh)

    def conv(x, w):
        return np.einsum("oc,bc...->bo...", w, x)

    def relu(x):
        return np.maximum(x, 0.0)

    Bb, _, Hh, Ww = features.shape
    h = relu(bn(conv(features, p["proj_w1"]), *p["proj_bn1"]))
    h = bn(conv(h, p["proj_w2"]), *p["proj_bn2"])
    s = bn(conv(features, p["proj_ws"]), *p["proj_bns"])
    x = relu(h + s)

    def unfold(t):
        tp = np.pad(t, ((0, 0), (0, 0), (1, 1), (1, 1)))
        return np.stack(
            [tp[:, :, di : di + Hh, dj : dj + Ww] for di in range(3) for dj in range(3)],
            axis=2,
        )

    feat_n = unfold(x)
    rel = unfold(cart) - cart[:, :, None]
    pe = relu(bn(conv(rel, p["pos_w1"]), *p["pos_bn1"]))
    pe = relu(bn(conv(pe, p["pos_w2"]), *p["pos_bn2"]))
    geo = (pe * feat_n).reshape(Bb, -1, Hh, Ww)
    geo = relu(bn(conv(geo, p["fus_w1"]), *p["fus_bn1"]))
    geo = relu(bn(conv(geo, p["fus_w2"]), *p["fus_bn2"]))
    return geo


def _assemble(results, features, cart, params):
    out = np.zeros((B, COUT, H, W), np.float32)
    for i, res in enumerate(results):
        o = res["out"]  # [128, LINT]
        j0 = i * WL
        a = o[0:64].reshape(64, HH, WP)[:, :, 1 : 1 + WL]
        b = o[64:128].reshape(64, HH, WP)[:, :, 1 : 1 + WL]
        out[0, :, 0:HH, j0 : j0 + WL] = a
        out[0, :, HH:H, j0 : j0 + WL] = b

    # fix the two global-edge columns (x at a padded column is nonzero
    # in-kernel; the true semantics zero it)
    features = np.asarray(features, np.float32)
    cart = np.asarray(cart, np.float32)
    lo = _np_reference(features[:, :, :, 0:3], cart[:, :, :, 0:3], params)
    out[0, :, :, 0] = lo[0, :, :, 0]
    hi = _np_reference(features[:, :, :, W - 3 : W], cart[:, :, :, W - 3 : W], params)
    out[0, :, :, W - 1] = hi[0, :, :, 2]
    return out


def run_hw(features, cart, params, trace=False):
    nc = _get_compiled()
    in_maps = _prep_in_maps(features, cart, params)
    res = bass_utils.run_bass_kernel_spmd(
        nc, in_maps, list(range(NCORES)), trace=trace
    )
    out = _assemble(res.results, features, cart, params)
    return out, res


def kernel(features, cart, params):
    out, _ = run_hw(features, cart, params, trace=False)
    return out


# ---------------------------------------------------------------------------
# timing support: the minimal axon env has no NTFF profile hook, so we time
# repeated device-resident executions and subtract a null-kernel baseline.


def _collect_io(nc):
    partition_name = nc.partition_id_tensor.name if nc.partition_id_tensor else None
    in_names, out_names, out_avals = [], [], []
    import jax

    for alloc in nc.m.functions[0].allocations:
        if not isinstance(alloc, mybir.MemoryLocationSet):
            continue
        name = alloc.memorylocations[0].name
        if alloc.kind == "ExternalInput":
            if name != partition_name:
                in_names.append(name)
        elif alloc.kind == "ExternalOutput":
            shape = tuple(alloc.tensor_shape)
            dtype = mybir.dt.np(alloc.dtype)
            out_names.append(name)
            out_avals.append(jax.core.ShapedArray(shape, dtype))
    return partition_name, in_names, out_names, out_avals


def _make_timed_callable(nc, in_maps, chain=1):
    import jax
    from jax.sharding import Mesh, NamedSharding, PartitionSpec
    from jax.experimental.shard_map import shard_map
    from concourse import bass2jax

    bass2jax.install_neuronx_cc_hook()
    partition_name, in_names, out_names, out_avals = _collect_io(nc)
    n_params = len(in_names)
    all_names = in_names + out_names
    if partition_name is not None:
        all_names.append(partition_name)

    def _body(*args):
        ins = list(args[:n_params])
        outs = list(args[n_params:])
        for _ in range(chain):
            operands = ins + outs
            if partition_name is not None:
                operands.append(bass2jax.partition_id_tensor())
            outs = list(
                bass2jax._bass_exec_p.bind(
                    *operands,
                    out_avals=tuple(out_avals),
                    in_names=tuple(all_names),
                    out_names=tuple(out_names),
                    lowering_input_output_aliases=(),
                    sim_require_finite=True,
                    sim_require_nnan=True,
                    nc=nc,
                )
            )
        return tuple(outs)

    n = len(in_maps)
    devices = jax.devices()[:n]
    mesh = Mesh(np.asarray(devices), ("core",))
    spec = PartitionSpec("core")
    n_outs = len(out_names)
    sharded = jax.jit(
        shard_map(
            _body,
            mesh=mesh,
            in_specs=(spec,) * (n_params + n_outs),
            out_specs=(spec,) * n_outs,
            check_rep=False,
        ),
        keep_unused=True,
    )
    concat_in = [
        np.concatenate([np.asarray(m[name]) for m in in_maps], axis=0)
        for name in in_names
    ]
    concat_zeros = [
        np.zeros((n * a.shape[0], *a.shape[1:]), a.dtype) for a in out_avals
    ]
    sh = NamedSharding(mesh, spec)
    dargs = [jax.device_put(a, sh) for a in concat_in + concat_zeros]

    def call():
        jax.block_until_ready(sharded(*dargs))

    return call


def _null_nc():
    nc = bacc.Bacc("TRN2", target_bir_lowering=False, debug=False)
    src = nc.dram_tensor("nsrc", [128, 16], FP32, kind="ExternalInput").ap()
    dst = nc.dram_tensor("nout", [128, 16], FP32, kind="ExternalOutput").ap()
    with tile.TileContext(nc) as tc:
        with tc.tile_pool(name="p", bufs=1) as pool:
            t = pool.tile([128, 16], FP32)
            nc.sync.dma_start(t[:], src)
            nc.sync.dma_start(dst, t[:])
    nc.compile()
    return nc


def _time_callable(call, reps=50):
    import time

    call()
    call()
    ts = []
    for _ in range(reps):
        t0 = time.perf_counter()
        call()
        ts.append(time.perf_counter() - t0)
    ts.sort()
    return ts[len(ts) // 2], ts[0]


def measure_exec_ns(features, cart, params, reps=40, r1=1, r2=25):
    """Per-iteration time via the wall-clock slope between two kernels
    that repeat the computation r1 and r2 times internally."""
    in_maps = _prep_in_maps(features, cart, params)
    c1 = _make_timed_callable(_get_compiled(repeat=r1), in_maps)
    m1, b1 = _time_callable(c1, reps)
    c2 = _make_timed_callable(_get_compiled(repeat=r2), in_maps)
    m2, b2 = _time_callable(c2, reps)
    med_slope = (m2 - m1) / (r2 - r1)
    best_slope = (b2 - b1) / (r2 - r1)
    print(
        f"[timing] rep{r1} med {m1*1e3:.2f}ms best {b1*1e3:.2f}ms | "
        f"rep{r2} med {m2*1e3:.2f}ms best {b2*1e3:.2f}ms | "
        f"slope med {med_slope*1e6:.1f}us best {best_slope*1e6:.1f}us"
    )
    return max(0.0, best_slope) * 1e9


# revision 33
# speedup vs baseline: 1.0769x; 1.0769x over previous
"""Trainium2 Bass kernel for a GNN message-passing block (3x3 unfold +
1x1 convs), sharded over 8 NeuronCores along the W (azimuth) axis.

Layout strategy ("pixel-split planes"):
  Every on-chip tensor is [128, L] where partitions 0-63 hold the 64
  channels for the first half of this core's rows (half A) and
  partitions 64-127 hold the same channels for the second half (half B).
  All 1x1 convs become matmuls with block-diagonal [128,128] weights so
  one instruction serves both halves; all elementwise ops run at the
  full 128-lane width.

Algebraic restructuring vs the reference:
  - BN folded into conv weights/bias on the host.
  - pos_w1 @ rel_k == shift_k(u) - u with u = pos_w1 @ cart (computed
    once instead of 9x).
  - center tap (k=4): pe1 = relu(pb1) is constant, so its fusion
    contribution folds into the weights: F1_4' = F1_4 * c4.
  - x >= 0 (relu output), so relu ordering around the pe2*x product is
    flexible.

Halo handling: the host passes each core its W-slice plus one halo
column per side (zeros at the global edges). The only values the kernel
cannot reproduce are the global-edge output columns (x at a padded
column is not zero after the biased projection), so the host recomputes
output columns 0 and W-1 in numpy and overwrites them.
"""

import numpy as np
import ml_dtypes

import concourse.bass as bass
import concourse.bacc as bacc
import concourse.mybir as mybir
import concourse.tile as tile
from concourse import bass_utils

FP32 = mybir.dt.float32
FP32R = mybir.dt.float32r
BF16 = mybir.dt.bfloat16
Relu = mybir.ActivationFunctionType.Relu
Alu = mybir.AluOpType

EPS = 1e-5
B, CIN, COUT, H, W = 1, 64, 64, 64, 2048
NCORES = 8
WL = W // NCORES          # 256 interior columns per core
WP = WL + 2               # 258 columns incl. 1-col halo each side
HH = H // 2               # 32 output rows per half
ROWS = HH + 2             # 34 plane rows (1 halo/pad row each side)
L = ROWS * WP             # 8772 flat plane length (image coords)
DOFF = 2                  # plane data offset (guard elements)
LP = L + 4                # padded plane length
INT0 = DOFF + WP          # first interior element (row 1, col 0)
LINT = HH * WP            # 8256 interior length
CHUNK_A = 1024            # stage-A chunk
CHUNK_B = 2048            # stage-B chunk

# neighbor taps (di, dj), excluding the center (0,0)
TAPS = [(di, dj) for di in (-1, 0, 1) for dj in (-1, 0, 1) if not (di == 0 and dj == 0)]

# staged output-store quarters (flushed as soon as their chunks complete)
OUT_FLUSH = [(0, 2048), (2048, 4096), (4096, 6144), (6144, LINT)]


def _fold(w, bn):
    g, b, m, v = [np.asarray(t, np.float64) for t in bn]
    s = g / np.sqrt(v + EPS)
    return (np.asarray(w, np.float64) * s[:, None]).astype(np.float32), (
        b - m * s
    ).astype(np.float32)


def _bd(w):
    """[O, C] conv weight -> [128, 128] block-diag lhsT ([K, M] layout)."""
    o, c = w.shape
    out = np.zeros((128, 128), np.float32)
    out[0:c, 0:o] = w.T
    out[64 : 64 + c, 64 : 64 + o] = w.T
    return out


def _prep_weights(params):
    p = params
    w1, b1 = _fold(p["proj_w1"], p["proj_bn1"])
    w2, b2 = _fold(p["proj_w2"], p["proj_bn2"])
    ws, bs = _fold(p["proj_ws"], p["proj_bns"])
    p1, pb1 = _fold(p["pos_w1"], p["pos_bn1"])
    p2, pb2 = _fold(p["pos_w2"], p["pos_bn2"])
    f1, fb1 = _fold(p["fus_w1"], p["fus_bn1"])
    f2, fb2 = _fold(p["fus_w2"], p["fus_bn2"])
    bx = (b2 + bs).astype(np.float32)

    # center tap constant: pe2_center = relu(p2 @ relu(pb1) + pb2)
    c4 = np.maximum(p2 @ np.maximum(pb1, 0.0) + pb2, 0.0).astype(np.float32)
    f1k = [f1[:, k::9].copy() for k in range(9)]
    f1k[4] = f1k[4] * c4[None, :]

    # wb: bf16 weights packed side by side [128, 15*128]:
    #   0=w1 1=w2 2=ws 3=p2 4..12=f1(k) 13=f2 14=p1
    mats = [w1, w2, ws, p2] + f1k + [f2, p1]
    wb = np.concatenate([_bd(m) for m in mats], axis=1).astype(ml_dtypes.bfloat16)
    biases = np.stack(
        [np.concatenate([b, b]) for b in (b1, bx, pb1, pb2, fb1, fb2)], axis=1
    ).astype(np.float32)  # [128, 6]
    return wb, biases


def _plane_pair(img, j0):
    """img [C, H, W] (numpy f32) -> [2*C?, ...] no: [C-part pair planes].

    Returns [128-ish rows, L] with rows 0:C = half-A plane, rows
    64:64+C = half-B plane, flattened [ROWS, WP] per half. Halo columns
    come from the global image with zero padding at the W edges; halo
    rows (A row 0, B row 33) are zero."""
    ch = img.shape[0]
    wide = np.zeros((ch, H, WP), np.float32)
    lo = max(j0 - 1, 0)
    hi = min(j0 + WL + 1, W)
    wide[:, :, lo - (j0 - 1) : hi - (j0 - 1)] = img[:, :, lo:hi]
    out = np.zeros((128, L), np.float32)
    a = np.zeros((ch, ROWS, WP), np.float32)
    a[:, 1:34, :] = wide[:, 0:33, :]
    b = np.zeros((ch, ROWS, WP), np.float32)
    b[:, 0:33, :] = wide[:, 31:64, :]
    out[0:ch] = a.reshape(ch, L)
    out[64 : 64 + ch] = b.reshape(ch, L)
    return out


def build_kernel(tc, repeat=1):
    """Emit the per-core kernel IR. DRAM tensors are declared here.

    repeat > 1 re-emits the whole computation (timing aid: per-iteration
    time = wall-clock slope between two repeat counts)."""
    nc = tc.nc
    fio = nc.dram_tensor("fio", [128, L], BF16, kind="ExternalInput").ap()
    cart6 = nc.dram_tensor("cart6", [6, L], BF16, kind="ExternalInput").ap()
    wb_d = nc.dram_tensor("wb", [128, 15 * 128], BF16, kind="ExternalInput").ap()
    bias_d = nc.dram_tensor("bias", [128, 6], FP32, kind="ExternalInput").ap()
    out_d = nc.dram_tensor("out", [128, LINT], FP32, kind="ExternalOutput").ap()

    from contextlib import ExitStack

    with ExitStack() as ctx:
        const = ctx.enter_context(tc.tile_pool(name="const", bufs=1))
        planes = ctx.enter_context(tc.tile_pool(name="planes", bufs=1))

        wb_t = const.tile([128, 15 * 128], BF16)
        nc.sync.dma_start(wb_t[:], wb_d)
        bias_t = const.tile([128, 6], FP32)
        nc.sync.dma_start(bias_t[:], bias_d)

        def WB(i):
            return wb_t[:, i * 128 : (i + 1) * 128]

        def BIAS(i):
            return bias_t[:, i : i + 1]

        u_pl = planes.tile([128, LP], BF16)
        u1_pl = planes.tile([128, LP], BF16)
        x_pl = planes.tile([128, LP], BF16)
        x1_pl = planes.tile([128, LP], BF16)
        f_pl = planes.tile([128, L], BF16)
        c_pl = planes.tile([128, L], BF16)
        o_pl = planes.tile([128, LINT], FP32)

        for _rep in range(repeat):
            _build_iteration(
                tc, ctx, fio, cart6, bias_t, WB, BIAS,
                u_pl, u1_pl, x_pl, x1_pl, f_pl, c_pl, o_pl, out_d,
            )


def _build_iteration(
    tc, ctx, fio, cart6, bias_t, WB, BIAS,
    u_pl, u1_pl, x_pl, x1_pl, f_pl, c_pl, o_pl, out_d,
):
    nc = tc.nc
    if True:
        # f input: few big DMAs (descriptor-count bound on the issuing seq)
        NQ = 4
        qs = [L * i // NQ for i in range(NQ + 1)]
        for i in range(NQ):
            nc.sync.dma_start(f_pl[:, qs[i] : qs[i + 1]], fio[:, qs[i] : qs[i + 1]])
        # cart plane: zero once, then land the 3 channels per half
        nc.gpsimd.memset(c_pl[:], 0.0)
        nc.sync.dma_start(c_pl[0:3, :], cart6[0:3, :])
        nc.sync.dma_start(c_pl[64:67, :], cart6[3:6, :])

        # ---- stage A: u = p1@cart ; x = relu(w2@relu(w1@f+b1) + ws@f + bx)
        # evac engine split: u + h1 on DVE (idle in stage A), x on ACT.
        with (
            tc.tile_pool(name="aio", bufs=3) as aio,
            tc.tile_pool(name="astg", bufs=3) as astg,
            tc.tile_pool(name="apsum", bufs=1, space="PSUM") as apsum,
        ):
            for c0 in range(0, L, CHUNK_A):
                n = min(CHUNK_A, L - c0)
                d = DOFF + c0
                slices = [(s, min(512, n - s)) for s in range(0, n, 512)]
                pu = apsum.tile([128, CHUNK_A], FP32, tag="pu")
                for s, sn in slices:
                    nc.tensor.matmul(
                        pu[:, s : s + sn], WB(14), c_pl[:, c0 + s : c0 + s + sn],
                        start=True, stop=True,
                    )
                nc.scalar.copy(u_pl[:, d : d + n], pu[:, 0:n])

                ph = apsum.tile([128, CHUNK_A], FP32, tag="ph")
                for s, sn in slices:
                    nc.tensor.matmul(
                        ph[:, s : s + sn], WB(0), f_pl[:, c0 + s : c0 + s + sn],
                        start=True, stop=True,
                    )
                h1_t = astg.tile([128, CHUNK_A], BF16, tag="h1")
                nc.vector.tensor_scalar(
                    h1_t[:, 0:n], ph[:, 0:n], BIAS(0), 0.0, Alu.add, Alu.max
                )

                px = apsum.tile([128, CHUNK_A], FP32, tag="px")
                for s, sn in slices:
                    nc.tensor.matmul(
                        px[:, s : s + sn], WB(1), h1_t[:, s : s + sn],
                        start=True, stop=False,
                    )
                    nc.tensor.matmul(
                        px[:, s : s + sn], WB(2), f_pl[:, c0 + s : c0 + s + sn],
                        start=False, stop=True,
                    )
                nc.scalar.activation(x_pl[:, d : d + n], px[:, 0:n], Relu, bias=BIAS(1))

        # guard elements + pad rows, then the +1-shifted copies
        for pl in (u_pl, x_pl):
            nc.vector.memset(pl[:, 0:DOFF], 0.0)
            nc.vector.memset(pl[:, LP - 2 : LP], 0.0)
        # zero x at the out-of-image pad rows (A: image row -1, B: row H)
        nc.vector.memset(x_pl[0:64, DOFF : DOFF + WP], 0.0)
        nc.vector.memset(x_pl[64:128, DOFF + L - WP : DOFF + L], 0.0)
        # shifted planes: s1[m] = s[m-1], split so stage B can start on
        # early quarters before stage A fully finishes
        NSPLIT = 4
        bnds = [1 + (LP - 1) * i // NSPLIT for i in range(NSPLIT + 1)]
        for s1_pl, s_pl in ((u1_pl, u_pl), (x1_pl, x_pl)):
            for bi in range(NSPLIT):
                lo, hi = bnds[bi], bnds[bi + 1]
                nc.vector.tensor_copy(s1_pl[:, lo:hi], s_pl[:, lo - 1 : hi - 1])
            nc.vector.memset(s1_pl[:, 0:1], 0.0)

        # ---- stage B
        def shifted(base, s1, c, n, di, dj):
            off = c + di * WP
            if dj == -1:
                return s1[:, off : off + n]
            if dj == 0:
                return base[:, off : off + n]
            return s1[:, off + 2 : off + 2 + n]

        with (
            tc.tile_pool(name="bstg", bufs=4) as bstg,
            tc.tile_pool(name="gpool", bufs=2) as gpool,
            tc.tile_pool(name="bpsum", bufs=2, space="PSUM") as bpsum,
            tc.tile_pool(name="bpsum1", bufs=1, space="PSUM") as bpsum1,
        ):
            for c0 in range(0, LINT, CHUNK_B):
                n = min(CHUNK_B, LINT - c0)
                c = INT0 + c0
                # psum subtiles cover 1024 each; elementwise ops span n
                subs = [(h, min(1024, n - h)) for h in range(0, n, 1024)]

                def halved(h0, hn):
                    return [(h0 + s, min(512, hn - s)) for s in range(0, hn, 512)]

                # pe1/pe2/g for the 8 non-center taps
                gs = []
                for ki, (di, dj) in enumerate(TAPS):
                    pe1 = bstg.tile([128, CHUNK_B], BF16, tag="pe1")
                    nc.vector.tensor_sub(
                        pe1[:, 0:n], shifted(u_pl, u1_pl, c, n, di, dj), u_pl[:, c : c + n]
                    )
                    eng = nc.gpsimd if ki in (1, 4, 6) else nc.vector
                    eng.tensor_scalar(
                        pe1[:, 0:n], pe1[:, 0:n], BIAS(2), 0.0, Alu.add, Alu.max
                    )
                    g = gpool.tile([128, CHUNK_B], BF16, tag=f"g{ki}")
                    for h0, hn in subs:
                        pp = bpsum.tile([128, 1024], FP32, tag="pe2")
                        for s, sn in halved(h0, hn):
                            nc.tensor.matmul(
                                pp[:, s - h0 : s - h0 + sn], WB(3), pe1[:, s : s + sn],
                                start=True, stop=True,
                            )
                        nc.scalar.activation(
                            g[:, h0 : h0 + hn], pp[:, 0:hn], Relu, bias=BIAS(3)
                        )
                    nc.vector.tensor_mul(
                        g[:, 0:n], g[:, 0:n], shifted(x_pl, x1_pl, c, n, di, dj)
                    )
                    gs.append(g)

                # fusion accumulation: center (folded) + 8 taps
                for h0, hn in subs:
                    acc = bpsum1.tile([128, 1024], FP32, tag=f"acc{h0 // 1024}")
                    for s, sn in halved(h0, hn):
                        nc.tensor.matmul(
                            acc[:, s - h0 : s - h0 + sn], WB(4 + 4),
                            x_pl[:, c + s : c + s + sn],
                            start=True, stop=False,
                        )
                    for ki, (di, dj) in enumerate(TAPS):
                        k = (di + 1) * 3 + (dj + 1)
                        for s, sn in halved(h0, hn):
                            nc.tensor.matmul(
                                acc[:, s - h0 : s - h0 + sn], WB(4 + k),
                                gs[ki][:, s : s + sn],
                                start=False, stop=(ki == len(TAPS) - 1),
                            )
                    geo = bstg.tile([128, 1024], BF16, tag="geo")
                    nc.scalar.activation(geo[:, 0:hn], acc[:, 0:hn], Relu, bias=BIAS(4))

                    po = bpsum.tile([128, 1024], FP32, tag="pe2")
                    for s, sn in halved(h0, hn):
                        nc.tensor.matmul(
                            po[:, s - h0 : s - h0 + sn], WB(13),
                            geo[:, s - h0 : s - h0 + sn],
                            start=True, stop=True,
                        )
                    nc.scalar.activation(
                        o_pl[:, c0 + h0 : c0 + h0 + hn], po[:, 0:hn],
                        Relu, bias=BIAS(5),
                    )
                # staged output DMA: flush completed quarters so the store
                # overlaps compute instead of serializing at the end
                done = c0 + n
                prev = c0
                for q0, q1 in OUT_FLUSH:
                    if prev < q1 <= done:
                        nc.sync.dma_start(out_d[:, q0:q1], o_pl[:, q0:q1])


_CACHE = {}


def _get_compiled(repeat=1):
    key = ("nc", repeat)
    if key not in _CACHE:
        nc = bacc.Bacc("TRN2", target_bir_lowering=False, debug=False)
        with tile.TileContext(nc) as tc:
            build_kernel(tc, repeat=repeat)
        nc.compile()
        _CACHE[key] = nc
    return _CACHE[key]


def _prep_in_maps(features, cart, params):
    features = np.asarray(features, np.float32)
    cart = np.asarray(cart, np.float32)
    wb, biases = _prep_weights(params)
    in_maps = []
    for i in range(NCORES):
        j0 = i * WL
        fio = _plane_pair(features[0], j0).astype(ml_dtypes.bfloat16)
        cp = _plane_pair(cart[0], j0)
        cart6 = (
            np.concatenate([cp[0:3], cp[64:67]], axis=0).astype(ml_dtypes.bfloat16)
        )
        in_maps.append({"fio": fio, "cart6": cart6, "wb": wb, "bias": biases})
    return in_maps


def _np_reference(features, cart, params):
    """Pure-numpy mirror of reference.reference (same zero-pad unfold)."""
    p = params

    def bn(x, g, b, m, v):
        sh = (1, -1) + (1,) * (x.ndim - 2)
        s = g / np.sqrt(v + EPS)
        return x * s.reshape(sh) + (b - m * s).reshape(sh)

    def conv(x, w):
        return np.einsum("oc,bc...->bo...", w, x)

    def relu(x):
        return np.maximum(x, 0.0)

    Bb, _, Hh, Ww = features.shape
    h = relu(bn(conv(features, p["proj_w1"]), *p["proj_bn1"]))
    h = bn(conv(h, p["proj_w2"]), *p["proj_bn2"])
    s = bn(conv(features, p["proj_ws"]), *p["proj_bns"])
    x = relu(h + s)

    def unfold(t):
        tp = np.pad(t, ((0, 0), (0, 0), (1, 1), (1, 1)))
        return np.stack(
            [tp[:, :, di : di + Hh, dj : dj + Ww] for di in range(3) for dj in range(3)],
            axis=2,
        )

    feat_n = unfold(x)
    rel = unfold(cart) - cart[:, :, None]
    pe = relu(bn(conv(rel, p["pos_w1"]), *p["pos_bn1"]))
    pe = relu(bn(conv(pe, p["pos_w2"]), *p["pos_bn2"]))
    geo = (pe * feat_n).reshape(Bb, -1, Hh, Ww)
    geo = relu(bn(conv(geo, p["fus_w1"]), *p["fus_bn1"]))
    geo = relu(bn(conv(geo, p["fus_w2"]), *p["fus_bn2"]))
    return geo


def _assemble(results, features, cart, params):
    out = np.zeros((B, COUT, H, W), np.float32)
    for i, res in enumerate(results):
        o = res["out"]  # [128, LINT]
        j0 = i * WL
        a = o[0:64].reshape(64, HH, WP)[:, :, 1 : 1 + WL]
        b = o[64:128].reshape(64, HH, WP)[:, :, 1 : 1 + WL]
        out[0, :, 0:HH, j0 : j0 + WL] = a
        out[0, :, HH:H, j0 : j0 + WL] = b

    # fix the two global-edge columns (x at a padded column is nonzero
    # in-kernel; the true semantics zero it)
    features = np.asarray(features, np.float32)
    cart = np.asarray(cart, np.float32)
    lo = _np_reference(features[:, :, :, 0:3], cart[:, :, :, 0:3], params)
    out[0, :, :, 0] = lo[0, :, :, 0]
    hi = _np_reference(features[:, :, :, W - 3 : W], cart[:, :, :, W - 3 : W], params)
    out[0, :, :, W - 1] = hi[0, :, :, 2]
    return out


def run_hw(features, cart, params, trace=False):
    nc = _get_compiled()
    in_maps = _prep_in_maps(features, cart, params)
    res = bass_utils.run_bass_kernel_spmd(
        nc, in_maps, list(range(NCORES)), trace=trace
    )
    out = _assemble(res.results, features, cart, params)
    return out, res


def kernel(features, cart, params):
    out, _ = run_hw(features, cart, params, trace=False)
    return out


# ---------------------------------------------------------------------------
# timing support: the minimal axon env has no NTFF profile hook, so we time
# repeated device-resident executions and subtract a null-kernel baseline.


def _collect_io(nc):
    partition_name = nc.partition_id_tensor.name if nc.partition_id_tensor else None
    in_names, out_names, out_avals = [], [], []
    import jax

    for alloc in nc.m.functions[0].allocations:
        if not isinstance(alloc, mybir.MemoryLocationSet):
            continue
        name = alloc.memorylocations[0].name
        if alloc.kind == "ExternalInput":
            if name != partition_name:
                in_names.append(name)
        elif alloc.kind == "ExternalOutput":
            shape = tuple(alloc.tensor_shape)
            dtype = mybir.dt.np(alloc.dtype)
            out_names.append(name)
            out_avals.append(jax.core.ShapedArray(shape, dtype))
    return partition_name, in_names, out_names, out_avals


def _make_timed_callable(nc, in_maps, chain=1):
    import jax
    from jax.sharding import Mesh, NamedSharding, PartitionSpec
    from jax.experimental.shard_map import shard_map
    from concourse import bass2jax

    bass2jax.install_neuronx_cc_hook()
    partition_name, in_names, out_names, out_avals = _collect_io(nc)
    n_params = len(in_names)
    all_names = in_names + out_names
    if partition_name is not None:
        all_names.append(partition_name)

    def _body(*args):
        ins = list(args[:n_params])
        outs = list(args[n_params:])
        for _ in range(chain):
            operands = ins + outs
            if partition_name is not None:
                operands.append(bass2jax.partition_id_tensor())
            outs = list(
                bass2jax._bass_exec_p.bind(
                    *operands,
                    out_avals=tuple(out_avals),
                    in_names=tuple(all_names),
                    out_names=tuple(out_names),
                    lowering_input_output_aliases=(),
                    sim_require_finite=True,
                    sim_require_nnan=True,
                    nc=nc,
                )
            )
        return tuple(outs)

    n = len(in_maps)
    devices = jax.devices()[:n]
    mesh = Mesh(np.asarray(devices), ("core",))
    spec = PartitionSpec("core")
    n_outs = len(out_names)
    sharded = jax.jit(
        shard_map(
            _body,
            mesh=mesh,
            in_specs=(spec,) * (n_params + n_outs),
            out_specs=(spec,) * n_outs,
            check_rep=False,
        ),
        keep_unused=True,
    )
    concat_in = [
        np.concatenate([np.asarray(m[name]) for m in in_maps], axis=0)
        for name in in_names
    ]
    concat_zeros = [
        np.zeros((n * a.shape[0], *a.shape[1:]), a.dtype) for a in out_avals
    ]
    sh = NamedSharding(mesh, spec)
    dargs = [jax.device_put(a, sh) for a in concat_in + concat_zeros]

    def call():
        jax.block_until_ready(sharded(*dargs))

    return call


def _null_nc():
    nc = bacc.Bacc("TRN2", target_bir_lowering=False, debug=False)
    src = nc.dram_tensor("nsrc", [128, 16], FP32, kind="ExternalInput").ap()
    dst = nc.dram_tensor("nout", [128, 16], FP32, kind="ExternalOutput").ap()
    with tile.TileContext(nc) as tc:
        with tc.tile_pool(name="p", bufs=1) as pool:
            t = pool.tile([128, 16], FP32)
            nc.sync.dma_start(t[:], src)
            nc.sync.dma_start(dst, t[:])
    nc.compile()
    return nc


def measure_exec_ns(features, cart, params, reps=30, rlist=(1, 13, 25)):
    """Per-iteration time via interleaved pair slopes between kernels with
    different internal repeat counts (cancels tunnel-latency drift)."""
    import time

    in_maps = _prep_in_maps(features, cart, params)
    calls = [
        (r, _make_timed_callable(_get_compiled(repeat=r), in_maps)) for r in rlist
    ]
    for _, c in calls:
        c()
        c()
    samples = {r: [] for r, _ in calls}
    for i in range(reps):
        for r, c in calls:
            t0 = time.perf_counter()
            c()
            samples[r].append(time.perf_counter() - t0)

    def med(v):
        v = sorted(v)
        return v[len(v) // 2]

    meds = {r: med(v) for r, v in samples.items()}
    slopes = []
    rs = list(rlist)
    for a, b in zip(rs[:-1], rs[1:]):
        slopes.append((meds[b] - meds[a]) / (b - a))
    # pairwise-interleaved slope between the extremes (drift-robust)
    pair = [
        (samples[rs[-1]][i] - samples[rs[0]][i]) / (rs[-1] - rs[0])
        for i in range(reps)
    ]
    pslope = med(pair)
    print(
        "[timing] "
        + " | ".join(f"rep{r} med {meds[r]*1e3:.2f}ms" for r in rs)
        + " | seg slopes "
        + ", ".join(f"{s*1e6:.1f}us" for s in slopes)
        + f" | paired slope {pslope*1e6:.1f}us"
    )
    return max(0.0, pslope) * 1e9


# revision 39
# speedup vs baseline: 1.5779x; 1.4652x over previous
"""Trainium2 Bass kernel for a GNN message-passing block (3x3 unfold +
1x1 convs), sharded over 8 NeuronCores along the W (azimuth) axis.

Layout strategy ("pixel-split planes"):
  Every on-chip tensor is [128, L] where partitions 0-63 hold the 64
  channels for the first half of this core's rows (half A) and
  partitions 64-127 hold the same channels for the second half (half B).
  All 1x1 convs become matmuls with block-diagonal [128,128] weights so
  one instruction serves both halves; all elementwise ops run at the
  full 128-lane width.

Algebraic restructuring vs the reference:
  - BN folded into conv weights/bias on the host.
  - pos_w1 @ rel_k == shift_k(u) - u with u = pos_w1 @ cart (computed
    once instead of 9x).
  - center tap (k=4): pe1 = relu(pb1) is constant, so its fusion
    contribution folds into the weights: F1_4' = F1_4 * c4.
  - x >= 0 (relu output), so relu ordering around the pe2*x product is
    flexible.

Halo handling: the host passes each core its W-slice plus one halo
column per side (zeros at the global edges). The only values the kernel
cannot reproduce are the global-edge output columns (x at a padded
column is not zero after the biased projection), so the host recomputes
output columns 0 and W-1 in numpy and overwrites them.
"""

import numpy as np
import ml_dtypes

import concourse.bass as bass
import concourse.bacc as bacc
import concourse.mybir as mybir
import concourse.tile as tile
from concourse import bass_utils

FP32 = mybir.dt.float32
FP32R = mybir.dt.float32r
BF16 = mybir.dt.bfloat16
Relu = mybir.ActivationFunctionType.Relu
Alu = mybir.AluOpType

EPS = 1e-5
B, CIN, COUT, H, W = 1, 64, 64, 64, 2048
NCORES = 8
WL = W // NCORES          # 256 interior columns per core
WP = WL + 2               # 258 columns incl. 1-col halo each side
HH = H // 2               # 32 output rows per half
ROWS = HH + 2             # 34 plane rows (1 halo/pad row each side)
L = ROWS * WP             # 8772 flat plane length (image coords)
DOFF = 2                  # plane data offset (guard elements)
LP = L + 4                # padded plane length
INT0 = DOFF + WP          # first interior element (row 1, col 0)
LINT = HH * WP            # 8256 interior length
CHUNK_A = 1024            # stage-A chunk
CHUNK_B = 2048            # stage-B chunk

# neighbor taps (di, dj), excluding the center (0,0)
TAPS = [(di, dj) for di in (-1, 0, 1) for dj in (-1, 0, 1) if not (di == 0 and dj == 0)]

# ablation switch for hardware timing decomposition (timing only -- results
# are garbage for any value other than "")
ABLATE = ""

# tap pairs for the strided-AP trick: (k_first, k_second, use_shift1_plane,
# base_delta, outer_step) -- both taps of a pair read the same (u or u1)
# plane at two windows separated by outer_step
TAP_PAIRS = [
    (0, 6, True, -WP, 2 * WP),   # dj=-1: (-1,-1) & (+1,-1)
    (1, 7, False, -WP, 2 * WP),  # dj= 0: (-1,0) & (+1,0)
    (2, 8, True, 2 - WP, 2 * WP),  # dj=+1: (-1,+1) & (+1,+1)
    (3, 5, True, 0, 2),          # di= 0: (0,-1) & (0,+1)
]

# staged output-store quarters (flushed as soon as their chunks complete)
OUT_FLUSH = [(0, 2048), (2048, 4096), (4096, 6144), (6144, LINT)]


def _fold(w, bn):
    g, b, m, v = [np.asarray(t, np.float64) for t in bn]
    s = g / np.sqrt(v + EPS)
    return (np.asarray(w, np.float64) * s[:, None]).astype(np.float32), (
        b - m * s
    ).astype(np.float32)


def _bd(w):
    """[O, C] conv weight -> [128, 128] block-diag lhsT ([K, M] layout)."""
    o, c = w.shape
    out = np.zeros((128, 128), np.float32)
    out[0:c, 0:o] = w.T
    out[64 : 64 + c, 64 : 64 + o] = w.T
    return out


def _prep_weights(params):
    p = params
    w1, b1 = _fold(p["proj_w1"], p["proj_bn1"])
    w2, b2 = _fold(p["proj_w2"], p["proj_bn2"])
    ws, bs = _fold(p["proj_ws"], p["proj_bns"])
    p1, pb1 = _fold(p["pos_w1"], p["pos_bn1"])
    p2, pb2 = _fold(p["pos_w2"], p["pos_bn2"])
    f1, fb1 = _fold(p["fus_w1"], p["fus_bn1"])
    f2, fb2 = _fold(p["fus_w2"], p["fus_bn2"])
    bx = (b2 + bs).astype(np.float32)

    # center tap constant: pe2_center = relu(p2 @ relu(pb1) + pb2)
    c4 = np.maximum(p2 @ np.maximum(pb1, 0.0) + pb2, 0.0).astype(np.float32)
    f1k = [f1[:, k::9].copy() for k in range(9)]
    f1k[4] = f1k[4] * c4[None, :]

    # wb: bf16 weights packed side by side [128, 15*128]:
    #   0=w1 1=w2 2=ws 3=p2 4..12=f1(k) 13=f2 14=p1
    mats = [w1, w2, ws, p2] + f1k + [f2, p1]
    wb = np.concatenate([_bd(m) for m in mats], axis=1).astype(ml_dtypes.bfloat16)
    biases = np.stack(
        [np.concatenate([b, b]) for b in (b1, bx, pb1, pb2, fb1, fb2)], axis=1
    ).astype(np.float32)  # [128, 6]
    return wb, biases


def _plane_pair(img, j0):
    """img [C, H, W] (numpy f32) -> [2*C?, ...] no: [C-part pair planes].

    Returns [128-ish rows, L] with rows 0:C = half-A plane, rows
    64:64+C = half-B plane, flattened [ROWS, WP] per half. Halo columns
    come from the global image with zero padding at the W edges; halo
    rows (A row 0, B row 33) are zero."""
    ch = img.shape[0]
    wide = np.zeros((ch, H, WP), np.float32)
    lo = max(j0 - 1, 0)
    hi = min(j0 + WL + 1, W)
    wide[:, :, lo - (j0 - 1) : hi - (j0 - 1)] = img[:, :, lo:hi]
    out = np.zeros((128, L), np.float32)
    a = np.zeros((ch, ROWS, WP), np.float32)
    a[:, 1:34, :] = wide[:, 0:33, :]
    b = np.zeros((ch, ROWS, WP), np.float32)
    b[:, 0:33, :] = wide[:, 31:64, :]
    out[0:ch] = a.reshape(ch, L)
    out[64 : 64 + ch] = b.reshape(ch, L)
    return out


def build_kernel(tc, repeat=1):
    """Emit the per-core kernel IR. DRAM tensors are declared here.

    repeat > 1 re-emits the whole computation (timing aid: per-iteration
    time = wall-clock slope between two repeat counts)."""
    nc = tc.nc
    fio = nc.dram_tensor("fio", [128, L], BF16, kind="ExternalInput").ap()
    cart6 = nc.dram_tensor("cart6", [6, L], BF16, kind="ExternalInput").ap()
    wb_d = nc.dram_tensor("wb", [128, 15 * 128], BF16, kind="ExternalInput").ap()
    bias_d = nc.dram_tensor("bias", [128, 6], FP32, kind="ExternalInput").ap()
    out_d = nc.dram_tensor("out", [128, LINT], FP32, kind="ExternalOutput").ap()

    from contextlib import ExitStack

    with ExitStack() as ctx:
        const = ctx.enter_context(tc.tile_pool(name="const", bufs=1))
        planes = ctx.enter_context(tc.tile_pool(name="planes", bufs=1))

        wb_t = const.tile([128, 15 * 128], BF16)
        nc.sync.dma_start(wb_t[:], wb_d)
        bias_t = const.tile([128, 6], FP32)
        nc.sync.dma_start(bias_t[:], bias_d)

        def WB(i):
            return wb_t[:, i * 128 : (i + 1) * 128]

        def BIAS(i):
            return bias_t[:, i : i + 1]

        u_pl = planes.tile([128, LP], BF16)
        u1_pl = planes.tile([128, LP], BF16)
        x_pl = planes.tile([128, LP], BF16)
        x1_pl = planes.tile([128, LP], BF16)
        o_pl = planes.tile([128, LINT], FP32)

        for _rep in range(repeat):
            _build_iteration(
                tc, ctx, fio, cart6, bias_t, WB, BIAS,
                u_pl, u1_pl, x_pl, x1_pl, o_pl, out_d,
            )


def _build_iteration(
    tc, ctx, fio, cart6, bias_t, WB, BIAS,
    u_pl, u1_pl, x_pl, x1_pl, o_pl, out_d,
):
    nc = tc.nc
    if True:

        # ---- stage A: u = p1@cart ; x = relu(w2@relu(w1@f+b1) + ws@f + bx)
        # evac engine split: u + h1 on DVE (idle in stage A), x on ACT.
        with (
            tc.tile_pool(name="aio", bufs=3) as aio,
            tc.tile_pool(name="astg", bufs=3) as astg,
            tc.tile_pool(name="apsum", bufs=1, space="PSUM") as apsum,
        ):
            for c0 in range(0, L, CHUNK_A):
                n = min(CHUNK_A, L - c0)
                d = DOFF + c0
                slices = [(s, min(512, n - s)) for s in range(0, n, 512)]
                f_t = aio.tile([128, CHUNK_A], BF16, tag="f")
                nc.sync.dma_start(f_t[:, 0:n], fio[:, c0 : c0 + n])
                cart_t = aio.tile([128, CHUNK_A], BF16, tag="cart")
                nc.gpsimd.memset(cart_t[:, 0:n], 0.0)
                nc.sync.dma_start(cart_t[0:3, 0:n], cart6[0:3, c0 : c0 + n])
                nc.sync.dma_start(cart_t[64:67, 0:n], cart6[3:6, c0 : c0 + n])
                pu = apsum.tile([128, CHUNK_A], FP32, tag="pu")
                for s, sn in slices:
                    nc.tensor.matmul(
                        pu[:, s : s + sn], WB(14), cart_t[:, s : s + sn],
                        start=True, stop=True,
                    )
                nc.scalar.copy(u_pl[:, d : d + n], pu[:, 0:n])

                ph = apsum.tile([128, CHUNK_A], FP32, tag="ph")
                for s, sn in slices:
                    nc.tensor.matmul(
                        ph[:, s : s + sn], WB(0), f_t[:, s : s + sn],
                        start=True, stop=True,
                    )
                h1_t = astg.tile([128, CHUNK_A], BF16, tag="h1")
                nc.vector.tensor_scalar(
                    h1_t[:, 0:n], ph[:, 0:n], BIAS(0), 0.0, Alu.add, Alu.max
                )

                px = apsum.tile([128, CHUNK_A], FP32, tag="px")
                for s, sn in slices:
                    nc.tensor.matmul(
                        px[:, s : s + sn], WB(1), h1_t[:, s : s + sn],
                        start=True, stop=False,
                    )
                    nc.tensor.matmul(
                        px[:, s : s + sn], WB(2), f_t[:, s : s + sn],
                        start=False, stop=True,
                    )
                nc.scalar.activation(x_pl[:, d : d + n], px[:, 0:n], Relu, bias=BIAS(1))

        # guard elements + pad rows, then the +1-shifted copies
        for pl in (u_pl, x_pl):
            nc.vector.memset(pl[:, 0:DOFF], 0.0)
            nc.vector.memset(pl[:, LP - 2 : LP], 0.0)
        # zero x at the out-of-image pad rows (A: image row -1, B: row H)
        nc.vector.memset(x_pl[0:64, DOFF : DOFF + WP], 0.0)
        nc.vector.memset(x_pl[64:128, DOFF + L - WP : DOFF + L], 0.0)
        # shifted planes: s1[m] = s[m-1], split so stage B can start on
        # early quarters before stage A fully finishes
        NSPLIT = 4
        bnds = [1 + (LP - 1) * i // NSPLIT for i in range(NSPLIT + 1)]
        for s1_pl, s_pl in ((u1_pl, u_pl), (x1_pl, x_pl)):
            for bi in range(NSPLIT):
                lo, hi = bnds[bi], bnds[bi + 1]
                nc.vector.tensor_copy(s1_pl[:, lo:hi], s_pl[:, lo - 1 : hi - 1])
            nc.vector.memset(s1_pl[:, 0:1], 0.0)

        # ---- stage B
        def shifted(base, s1, c, n, di, dj):
            off = c + di * WP
            if dj == -1:
                return s1[:, off : off + n]
            if dj == 0:
                return base[:, off : off + n]
            return s1[:, off + 2 : off + 2 + n]

        with (
            tc.tile_pool(name="bstg", bufs=2) as bstg,
            tc.tile_pool(name="gpool", bufs=2) as gpool,
            tc.tile_pool(name="bpsum", bufs=2, space="PSUM") as bpsum,
            tc.tile_pool(name="bpsum1", bufs=1, space="PSUM") as bpsum1,
        ):
            for c0 in range(0, LINT, CHUNK_B):
                n = min(CHUNK_B, LINT - c0)
                c = INT0 + c0
                # psum subtiles cover 1024 each; elementwise ops span n
                subs = [(h, min(1024, n - h)) for h in range(0, n, 1024)]

                def halved(h0, hn):
                    return [(h0 + s, min(512, hn - s)) for s in range(0, hn, 512)]

                # taps processed as 4 +/- pairs: one strided 3-level AP
                # reads both shifted windows in a single DVE instruction
                def pair_ap(pl, delta, step):
                    base = pl[:, c + delta : c + delta + n]
                    return bass.AP(
                        base.tensor, base.offset,
                        [list(base.ap[0]), [step, 2], [1, n]],
                    )

                gmap = {}
                for pi, (ka, kb, use1, delta, step) in enumerate(TAP_PAIRS):
                    pe1 = bstg.tile([128, 2 * CHUNK_B], BF16, tag="pe1")
                    if ABLATE != "nosub":
                        nc.vector.tensor_sub(
                            pe1[:, 0 : 2 * n],
                            pair_ap(u1_pl if use1 else u_pl, delta, step),
                            pair_ap(u_pl, 0, 0),
                        )
                        eng = nc.gpsimd if pi == 1 else nc.vector
                        eng.tensor_scalar(
                            pe1[:, 0 : 2 * n], pe1[:, 0 : 2 * n],
                            BIAS(2), 0.0, Alu.add, Alu.max,
                        )
                    g = gpool.tile([128, 2 * CHUNK_B], BF16, tag=f"gp{pi}")
                    for half, kk in enumerate((ka, kb)):
                        off = half * n
                        for h0, hn in subs:
                            pp = bpsum.tile([128, 1024], FP32, tag="pe2")
                            if ABLATE != "nomm":
                                for s, sn in halved(h0, hn):
                                    nc.tensor.matmul(
                                        pp[:, s - h0 : s - h0 + sn], WB(3),
                                        pe1[:, off + s : off + s + sn],
                                        start=True, stop=True,
                                    )
                            if ABLATE != "noevac":
                                nc.scalar.activation(
                                    g[:, off + h0 : off + h0 + hn], pp[:, 0:hn],
                                    Relu, bias=BIAS(3),
                                )
                        gmap[kk] = (g, half)
                    if ABLATE != "nomul":
                        nc.vector.tensor_mul(
                            g[:, 0 : 2 * n], g[:, 0 : 2 * n],
                            pair_ap(x1_pl if use1 else x_pl, delta, step),
                        )

                # fusion accumulation: center (folded) + 8 taps
                accs = []
                for h0, hn in subs:
                    acc_t = bpsum1.tile([128, 1024], FP32, tag=f"acc{h0 // 1024}")
                    accs.append(acc_t)
                if ABLATE != "nomm":
                    # k-outer order: each F1_k weight is loaded once per chunk
                    for hi, (h0, hn) in enumerate(subs):
                        for s, sn in halved(h0, hn):
                            nc.tensor.matmul(
                                accs[hi][:, s - h0 : s - h0 + sn], WB(4 + 4),
                                x_pl[:, c + s : c + s + sn],
                                start=True, stop=False,
                            )
                    klist = sorted(gmap)
                    for kn, k in enumerate(klist):
                        g, half = gmap[k]
                        goff = half * n
                        for hi, (h0, hn) in enumerate(subs):
                            for s, sn in halved(h0, hn):
                                nc.tensor.matmul(
                                    accs[hi][:, s - h0 : s - h0 + sn], WB(4 + k),
                                    g[:, goff + s : goff + s + sn],
                                    start=False, stop=(kn == len(klist) - 1),
                                )
                for hi, (h0, hn) in enumerate(subs):
                    acc = accs[hi]
                    geo = bstg.tile([128, 1024], BF16, tag="geo")
                    if ABLATE != "noevac":
                        nc.scalar.activation(
                            geo[:, 0:hn], acc[:, 0:hn], Relu, bias=BIAS(4)
                        )
                    po = bpsum.tile([128, 1024], FP32, tag="pe2")
                    if ABLATE != "nomm":
                        for s, sn in halved(h0, hn):
                            nc.tensor.matmul(
                                po[:, s - h0 : s - h0 + sn], WB(13),
                                geo[:, s - h0 : s - h0 + sn],
                                start=True, stop=True,
                            )
                    if ABLATE != "noevac":
                        nc.scalar.activation(
                            o_pl[:, c0 + h0 : c0 + h0 + hn], po[:, 0:hn],
                            Relu, bias=BIAS(5),
                        )
                # staged output DMA: flush completed quarters so the store
                # overlaps compute instead of serializing at the end
                done = c0 + n
                prev = c0
                for q0, q1 in OUT_FLUSH:
                    if prev < q1 <= done:
                        nc.sync.dma_start(out_d[:, q0:q1], o_pl[:, q0:q1])


_CACHE = {}


def _get_compiled(repeat=1):
    key = ("nc", repeat)
    if key not in _CACHE:
        nc = bacc.Bacc("TRN2", target_bir_lowering=False, debug=False)
        with tile.TileContext(nc) as tc:
            build_kernel(tc, repeat=repeat)
        nc.compile()
        _CACHE[key] = nc
    return _CACHE[key]


def _prep_in_maps(features, cart, params):
    features = np.asarray(features, np.float32)
    cart = np.asarray(cart, np.float32)
    wb, biases = _prep_weights(params)
    in_maps = []
    for i in range(NCORES):
        j0 = i * WL
        fio = _plane_pair(features[0], j0).astype(ml_dtypes.bfloat16)
        cp = _plane_pair(cart[0], j0)
        cart6 = (
            np.concatenate([cp[0:3], cp[64:67]], axis=0).astype(ml_dtypes.bfloat16)
        )
        in_maps.append({"fio": fio, "cart6": cart6, "wb": wb, "bias": biases})
    return in_maps


def _np_reference(features, cart, params):
    """Pure-numpy mirror of reference.reference (same zero-pad unfold)."""
    p = params

    def bn(x, g, b, m, v):
        sh = (1, -1) + (1,) * (x.ndim - 2)
        s = g / np.sqrt(v + EPS)
        return x * s.reshape(sh) + (b - m * s).reshape(sh)

    def conv(x, w):
        return np.einsum("oc,bc...->bo...", w, x)

    def relu(x):
        return np.maximum(x, 0.0)

    Bb, _, Hh, Ww = features.shape
    h = relu(bn(conv(features, p["proj_w1"]), *p["proj_bn1"]))
    h = bn(conv(h, p["proj_w2"]), *p["proj_bn2"])
    s = bn(conv(features, p["proj_ws"]), *p["proj_bns"])
    x = relu(h + s)

    def unfold(t):
        tp = np.pad(t, ((0, 0), (0, 0), (1, 1), (1, 1)))
        return np.stack(
            [tp[:, :, di : di + Hh, dj : dj + Ww] for di in range(3) for dj in range(3)],
            axis=2,
        )

    feat_n = unfold(x)
    rel = unfold(cart) - cart[:, :, None]
    pe = relu(bn(conv(rel, p["pos_w1"]), *p["pos_bn1"]))
    pe = relu(bn(conv(pe, p["pos_w2"]), *p["pos_bn2"]))
    geo = (pe * feat_n).reshape(Bb, -1, Hh, Ww)
    geo = relu(bn(conv(geo, p["fus_w1"]), *p["fus_bn1"]))
    geo = relu(bn(conv(geo, p["fus_w2"]), *p["fus_bn2"]))
    return geo


def _assemble(results, features, cart, params):
    out = np.zeros((B, COUT, H, W), np.float32)
    for i, res in enumerate(results):
        o = res["out"]  # [128, LINT]
        j0 = i * WL
        a = o[0:64].reshape(64, HH, WP)[:, :, 1 : 1 + WL]
        b = o[64:128].reshape(64, HH, WP)[:, :, 1 : 1 + WL]
        out[0, :, 0:HH, j0 : j0 + WL] = a
        out[0, :, HH:H, j0 : j0 + WL] = b

    # fix the two global-edge columns (x at a padded column is nonzero
    # in-kernel; the true semantics zero it)
    features = np.asarray(features, np.float32)
    cart = np.asarray(cart, np.float32)
    lo = _np_reference(features[:, :, :, 0:3], cart[:, :, :, 0:3], params)
    out[0, :, :, 0] = lo[0, :, :, 0]
    hi = _np_reference(features[:, :, :, W - 3 : W], cart[:, :, :, W - 3 : W], params)
    out[0, :, :, W - 1] = hi[0, :, :, 2]
    return out


def run_hw(features, cart, params, trace=False):
    nc = _get_compiled()
    in_maps = _prep_in_maps(features, cart, params)
    res = bass_utils.run_bass_kernel_spmd(
        nc, in_maps, list(range(NCORES)), trace=trace
    )
    out = _assemble(res.results, features, cart, params)
    return out, res


def kernel(features, cart, params):
    out, _ = run_hw(features, cart, params, trace=False)
    return out


# ---------------------------------------------------------------------------
# timing support: the minimal axon env has no NTFF profile hook, so we time
# repeated device-resident executions and subtract a null-kernel baseline.


def _collect_io(nc):
    partition_name = nc.partition_id_tensor.name if nc.partition_id_tensor else None
    in_names, out_names, out_avals = [], [], []
    import jax

    for alloc in nc.m.functions[0].allocations:
        if not isinstance(alloc, mybir.MemoryLocationSet):
            continue
        name = alloc.memorylocations[0].name
        if alloc.kind == "ExternalInput":
            if name != partition_name:
                in_names.append(name)
        elif alloc.kind == "ExternalOutput":
            shape = tuple(alloc.tensor_shape)
            dtype = mybir.dt.np(alloc.dtype)
            out_names.append(name)
            out_avals.append(jax.core.ShapedArray(shape, dtype))
    return partition_name, in_names, out_names, out_avals


def _make_timed_callable(nc, in_maps, chain=1):
    import jax
    from jax.sharding import Mesh, NamedSharding, PartitionSpec
    from jax.experimental.shard_map import shard_map
    from concourse import bass2jax

    bass2jax.install_neuronx_cc_hook()
    partition_name, in_names, out_names, out_avals = _collect_io(nc)
    n_params = len(in_names)
    all_names = in_names + out_names
    if partition_name is not None:
        all_names.append(partition_name)

    def _body(*args):
        ins = list(args[:n_params])
        outs = list(args[n_params:])
        for _ in range(chain):
            operands = ins + outs
            if partition_name is not None:
                operands.append(bass2jax.partition_id_tensor())
            outs = list(
                bass2jax._bass_exec_p.bind(
                    *operands,
                    out_avals=tuple(out_avals),
                    in_names=tuple(all_names),
                    out_names=tuple(out_names),
                    lowering_input_output_aliases=(),
                    sim_require_finite=True,
                    sim_require_nnan=True,
                    nc=nc,
                )
            )
        return tuple(outs)

    n = len(in_maps)
    devices = jax.devices()[:n]
    mesh = Mesh(np.asarray(devices), ("core",))
    spec = PartitionSpec("core")
    n_outs = len(out_names)
    sharded = jax.jit(
        shard_map(
            _body,
            mesh=mesh,
            in_specs=(spec,) * (n_params + n_outs),
            out_specs=(spec,) * n_outs,
            check_rep=False,
        ),
        keep_unused=True,
    )
    concat_in = [
        np.concatenate([np.asarray(m[name]) for m in in_maps], axis=0)
        for name in in_names
    ]
    concat_zeros = [
        np.zeros((n * a.shape[0], *a.shape[1:]), a.dtype) for a in out_avals
    ]
    sh = NamedSharding(mesh, spec)
    dargs = [jax.device_put(a, sh) for a in concat_in + concat_zeros]

    def call():
        jax.block_until_ready(sharded(*dargs))

    return call


def _null_nc():
    nc = bacc.Bacc("TRN2", target_bir_lowering=False, debug=False)
    src = nc.dram_tensor("nsrc", [128, 16], FP32, kind="ExternalInput").ap()
    dst = nc.dram_tensor("nout", [128, 16], FP32, kind="ExternalOutput").ap()
    with tile.TileContext(nc) as tc:
        with tc.tile_pool(name="p", bufs=1) as pool:
            t = pool.tile([128, 16], FP32)
            nc.sync.dma_start(t[:], src)
            nc.sync.dma_start(dst, t[:])
    nc.compile()
    return nc


def measure_exec_ns(features, cart, params, reps=30, rlist=(1, 13, 25)):
    """Per-iteration time via interleaved pair slopes between kernels with
    different internal repeat counts (cancels tunnel-latency drift)."""
    import time

    in_maps = _prep_in_maps(features, cart, params)
    calls = [
        (r, _make_timed_callable(_get_compiled(repeat=r), in_maps)) for r in rlist
    ]
    for _, c in calls:
        c()
        c()
    samples = {r: [] for r, _ in calls}
    for i in range(reps):
        for r, c in calls:
            t0 = time.perf_counter()
            c()
            samples[r].append(time.perf_counter() - t0)

    def med(v):
        v = sorted(v)
        return v[len(v) // 2]

    meds = {r: med(v) for r, v in samples.items()}
    slopes = []
    rs = list(rlist)
    for a, b in zip(rs[:-1], rs[1:]):
        slopes.append((meds[b] - meds[a]) / (b - a))
    # pairwise-interleaved slope between the extremes (drift-robust)
    pair = [
        (samples[rs[-1]][i] - samples[rs[0]][i]) / (rs[-1] - rs[0])
        for i in range(reps)
    ]
    pslope = med(pair)
    print(
        "[timing] "
        + " | ".join(f"rep{r} med {meds[r]*1e3:.2f}ms" for r in rs)
        + " | seg slopes "
        + ", ".join(f"{s*1e6:.1f}us" for s in slopes)
        + f" | paired slope {pslope*1e6:.1f}us"
    )
    return max(0.0, pslope) * 1e9


# revision 41
# speedup vs baseline: 1.6089x; 1.0197x over previous
"""Trainium2 Bass kernel for a GNN message-passing block (3x3 unfold +
1x1 convs), sharded over 8 NeuronCores along the W (azimuth) axis.

Layout strategy ("pixel-split planes"):
  Every on-chip tensor is [128, L] where partitions 0-63 hold the 64
  channels for the first half of this core's rows (half A) and
  partitions 64-127 hold the same channels for the second half (half B).
  All 1x1 convs become matmuls with block-diagonal [128,128] weights so
  one instruction serves both halves; all elementwise ops run at the
  full 128-lane width.

Algebraic restructuring vs the reference:
  - BN folded into conv weights/bias on the host.
  - pos_w1 @ rel_k == shift_k(u) - u with u = pos_w1 @ cart (computed
    once instead of 9x).
  - center tap (k=4): pe1 = relu(pb1) is constant, so its fusion
    contribution folds into the weights: F1_4' = F1_4 * c4.
  - x >= 0 (relu output), so relu ordering around the pe2*x product is
    flexible.

Halo handling: the host passes each core its W-slice plus one halo
column per side (zeros at the global edges). The only values the kernel
cannot reproduce are the global-edge output columns (x at a padded
column is not zero after the biased projection), so the host recomputes
output columns 0 and W-1 in numpy and overwrites them.
"""

import numpy as np
import ml_dtypes

import concourse.bass as bass
import concourse.bacc as bacc
import concourse.mybir as mybir
import concourse.tile as tile
from concourse import bass_utils

FP32 = mybir.dt.float32
FP32R = mybir.dt.float32r
BF16 = mybir.dt.bfloat16
Relu = mybir.ActivationFunctionType.Relu
Alu = mybir.AluOpType

EPS = 1e-5
B, CIN, COUT, H, W = 1, 64, 64, 64, 2048
NCORES = 8
WL = W // NCORES          # 256 interior columns per core
WP = WL + 2               # 258 columns incl. 1-col halo each side
HH = H // 2               # 32 output rows per half
ROWS = HH + 2             # 34 plane rows (1 halo/pad row each side)
L = ROWS * WP             # 8772 flat plane length (image coords)
DOFF = 2                  # plane data offset (guard elements)
LP = L + 4                # padded plane length
INT0 = DOFF + WP          # first interior element (row 1, col 0)
LINT = HH * WP            # 8256 interior length
CHUNK_A = 1024            # stage-A chunk
CHUNK_B = 2048            # stage-B chunk

# neighbor taps (di, dj), excluding the center (0,0)
TAPS = [(di, dj) for di in (-1, 0, 1) for dj in (-1, 0, 1) if not (di == 0 and dj == 0)]

# ablation switch for hardware timing decomposition (timing only -- results
# are garbage for any value other than "")
ABLATE = ""

# tap pairs for the strided-AP trick: (k_first, k_second, use_shift1_plane,
# base_delta, outer_step) -- both taps of a pair read the same (u or u1)
# plane at two windows separated by outer_step
TAP_PAIRS = [
    (0, 6, True, -WP, 2 * WP),   # dj=-1: (-1,-1) & (+1,-1)
    (1, 7, False, -WP, 2 * WP),  # dj= 0: (-1,0) & (+1,0)
    (2, 8, True, 2 - WP, 2 * WP),  # dj=+1: (-1,+1) & (+1,+1)
    (3, 5, True, 0, 2),          # di= 0: (0,-1) & (0,+1)
]

# staged output-store quarters (flushed as soon as their chunks complete)
OUT_FLUSH = [(0, 2048), (2048, 4096), (4096, 6144), (6144, LINT)]


def _fold(w, bn):
    g, b, m, v = [np.asarray(t, np.float64) for t in bn]
    s = g / np.sqrt(v + EPS)
    return (np.asarray(w, np.float64) * s[:, None]).astype(np.float32), (
        b - m * s
    ).astype(np.float32)


def _bd(w):
    """[O, C] conv weight -> [128, 128] block-diag lhsT ([K, M] layout)."""
    o, c = w.shape
    out = np.zeros((128, 128), np.float32)
    out[0:c, 0:o] = w.T
    out[64 : 64 + c, 64 : 64 + o] = w.T
    return out


def _prep_weights(params):
    p = params
    w1, b1 = _fold(p["proj_w1"], p["proj_bn1"])
    w2, b2 = _fold(p["proj_w2"], p["proj_bn2"])
    ws, bs = _fold(p["proj_ws"], p["proj_bns"])
    p1, pb1 = _fold(p["pos_w1"], p["pos_bn1"])
    p2, pb2 = _fold(p["pos_w2"], p["pos_bn2"])
    f1, fb1 = _fold(p["fus_w1"], p["fus_bn1"])
    f2, fb2 = _fold(p["fus_w2"], p["fus_bn2"])
    bx = (b2 + bs).astype(np.float32)

    # center tap constant: pe2_center = relu(p2 @ relu(pb1) + pb2)
    c4 = np.maximum(p2 @ np.maximum(pb1, 0.0) + pb2, 0.0).astype(np.float32)
    f1k = [f1[:, k::9].copy() for k in range(9)]
    f1k[4] = f1k[4] * c4[None, :]

    # wb: bf16 weights packed side by side [128, 15*128]:
    #   0=w1 1=w2 2=ws 3=p2 4..12=f1(k) 13=f2 14=p1
    mats = [w1, w2, ws, p2] + f1k + [f2, p1]
    wb = np.concatenate([_bd(m) for m in mats], axis=1).astype(ml_dtypes.bfloat16)
    biases = np.stack(
        [np.concatenate([b, b]) for b in (b1, bx, pb1, pb2, fb1, fb2)], axis=1
    ).astype(np.float32)  # [128, 6]
    return wb, biases


def _plane_pair(img, j0):
    """img [C, H, W] (numpy f32) -> [2*C?, ...] no: [C-part pair planes].

    Returns [128-ish rows, L] with rows 0:C = half-A plane, rows
    64:64+C = half-B plane, flattened [ROWS, WP] per half. Halo columns
    come from the global image with zero padding at the W edges; halo
    rows (A row 0, B row 33) are zero."""
    ch = img.shape[0]
    wide = np.zeros((ch, H, WP), np.float32)
    lo = max(j0 - 1, 0)
    hi = min(j0 + WL + 1, W)
    wide[:, :, lo - (j0 - 1) : hi - (j0 - 1)] = img[:, :, lo:hi]
    out = np.zeros((128, L), np.float32)
    a = np.zeros((ch, ROWS, WP), np.float32)
    a[:, 1:34, :] = wide[:, 0:33, :]
    b = np.zeros((ch, ROWS, WP), np.float32)
    b[:, 0:33, :] = wide[:, 31:64, :]
    out[0:ch] = a.reshape(ch, L)
    out[64 : 64 + ch] = b.reshape(ch, L)
    return out


def build_kernel(tc, repeat=1):
    """Emit the per-core kernel IR. DRAM tensors are declared here.

    repeat > 1 re-emits the whole computation (timing aid: per-iteration
    time = wall-clock slope between two repeat counts)."""
    nc = tc.nc
    fio = nc.dram_tensor("fio", [128, L], BF16, kind="ExternalInput").ap()
    cart6 = nc.dram_tensor("cart6", [6, L], BF16, kind="ExternalInput").ap()
    wb_d = nc.dram_tensor("wb", [128, 15 * 128], BF16, kind="ExternalInput").ap()
    bias_d = nc.dram_tensor("bias", [128, 6], FP32, kind="ExternalInput").ap()
    out_d = nc.dram_tensor("out", [128, LINT], FP32, kind="ExternalOutput").ap()

    from contextlib import ExitStack

    with ExitStack() as ctx:
        const = ctx.enter_context(tc.tile_pool(name="const", bufs=1))
        planes = ctx.enter_context(tc.tile_pool(name="planes", bufs=1))

        wb_t = const.tile([128, 15 * 128], BF16)
        nc.sync.dma_start(wb_t[:], wb_d)
        bias_t = const.tile([128, 6], FP32)
        nc.sync.dma_start(bias_t[:], bias_d)

        def WB(i):
            return wb_t[:, i * 128 : (i + 1) * 128]

        def BIAS(i):
            return bias_t[:, i : i + 1]

        u_pl = planes.tile([128, LP], BF16)
        u1_pl = planes.tile([128, LP], BF16)
        x_pl = planes.tile([128, LP], BF16)
        x1_pl = planes.tile([128, LP], BF16)
        o_pl = planes.tile([128, LINT], FP32)

        for _rep in range(repeat):
            _build_iteration(
                tc, ctx, fio, cart6, bias_t, WB, BIAS,
                u_pl, u1_pl, x_pl, x1_pl, o_pl, out_d,
            )


def _build_iteration(
    tc, ctx, fio, cart6, bias_t, WB, BIAS,
    u_pl, u1_pl, x_pl, x1_pl, o_pl, out_d,
):
    nc = tc.nc
    if True:

        # ---- stage A: u = p1@cart ; x = relu(w2@relu(w1@f+b1) + ws@f + bx)
        # evac engine split: u + h1 on DVE (idle in stage A), x on ACT.
        with (
            tc.tile_pool(name="aio", bufs=3) as aio,
            tc.tile_pool(name="astg", bufs=3) as astg,
            tc.tile_pool(name="apsum", bufs=1, space="PSUM") as apsum,
        ):
            for c0 in range(0, L, CHUNK_A):
                n = min(CHUNK_A, L - c0)
                d = DOFF + c0
                slices = [(s, min(512, n - s)) for s in range(0, n, 512)]
                f_t = aio.tile([128, CHUNK_A], BF16, tag="f")
                nc.sync.dma_start(f_t[:, 0:n], fio[:, c0 : c0 + n])
                cart_t = aio.tile([128, CHUNK_A], BF16, tag="cart")
                nc.gpsimd.memset(cart_t[:, 0:n], 0.0)
                nc.sync.dma_start(cart_t[0:3, 0:n], cart6[0:3, c0 : c0 + n])
                nc.sync.dma_start(cart_t[64:67, 0:n], cart6[3:6, c0 : c0 + n])
                pu = apsum.tile([128, CHUNK_A], FP32, tag="pu")
                for s, sn in slices:
                    nc.tensor.matmul(
                        pu[:, s : s + sn], WB(14), cart_t[:, s : s + sn],
                        start=True, stop=True,
                    )
                nc.scalar.copy(u_pl[:, d : d + n], pu[:, 0:n])

                ph = apsum.tile([128, CHUNK_A], FP32, tag="ph")
                for s, sn in slices:
                    nc.tensor.matmul(
                        ph[:, s : s + sn], WB(0), f_t[:, s : s + sn],
                        start=True, stop=True,
                    )
                h1_t = astg.tile([128, CHUNK_A], BF16, tag="h1")
                nc.vector.tensor_scalar(
                    h1_t[:, 0:n], ph[:, 0:n], BIAS(0), 0.0, Alu.add, Alu.max
                )

                px = apsum.tile([128, CHUNK_A], FP32, tag="px")
                for s, sn in slices:
                    nc.tensor.matmul(
                        px[:, s : s + sn], WB(1), h1_t[:, s : s + sn],
                        start=True, stop=False,
                    )
                    nc.tensor.matmul(
                        px[:, s : s + sn], WB(2), f_t[:, s : s + sn],
                        start=False, stop=True,
                    )
                nc.scalar.activation(x_pl[:, d : d + n], px[:, 0:n], Relu, bias=BIAS(1))

        # guard elements + pad rows, then the +1-shifted copies
        for pl in (u_pl, x_pl):
            nc.vector.memset(pl[:, 0:DOFF], 0.0)
            nc.vector.memset(pl[:, LP - 2 : LP], 0.0)
        # zero x at the out-of-image pad rows (A: image row -1, B: row H)
        nc.vector.memset(x_pl[0:64, DOFF : DOFF + WP], 0.0)
        nc.vector.memset(x_pl[64:128, DOFF + L - WP : DOFF + L], 0.0)
        # shifted planes: s1[m] = s[m-1], split so stage B can start on
        # early quarters before stage A fully finishes
        NSPLIT = 4
        bnds = [1 + (LP - 1) * i // NSPLIT for i in range(NSPLIT + 1)]
        for s1_pl, s_pl in ((u1_pl, u_pl), (x1_pl, x_pl)):
            for bi in range(NSPLIT):
                lo, hi = bnds[bi], bnds[bi + 1]
                nc.vector.tensor_copy(s1_pl[:, lo:hi], s_pl[:, lo - 1 : hi - 1])
            nc.vector.memset(s1_pl[:, 0:1], 0.0)

        # ---- stage B
        def shifted(base, s1, c, n, di, dj):
            off = c + di * WP
            if dj == -1:
                return s1[:, off : off + n]
            if dj == 0:
                return base[:, off : off + n]
            return s1[:, off + 2 : off + 2 + n]

        with (
            tc.tile_pool(name="bstg", bufs=2) as bstg,
            tc.tile_pool(name="gpool", bufs=2) as gpool,
            tc.tile_pool(name="bpsum", bufs=2, space="PSUM") as bpsum,
            tc.tile_pool(name="bpsum1", bufs=1, space="PSUM") as bpsum1,
        ):
            for c0 in range(0, LINT, CHUNK_B):
                n = min(CHUNK_B, LINT - c0)
                c = INT0 + c0
                # psum subtiles cover 1024 each; elementwise ops span n
                subs = [(h, min(1024, n - h)) for h in range(0, n, 1024)]

                def halved(h0, hn):
                    return [(h0 + s, min(512, hn - s)) for s in range(0, hn, 512)]

                # taps processed as 4 +/- pairs: one strided 3-level AP
                # reads both shifted windows in a single DVE instruction
                def pair_ap(pl, delta, step):
                    base = pl[:, c + delta : c + delta + n]
                    return bass.AP(
                        base.tensor, base.offset,
                        [list(base.ap[0]), [step, 2], [1, n]],
                    )

                gmap = {}
                for pi, (ka, kb, use1, delta, step) in enumerate(TAP_PAIRS):
                    pe1 = bstg.tile([128, 2 * CHUNK_B], BF16, tag="pe1")
                    if ABLATE != "nosub":
                        nc.vector.tensor_sub(
                            pe1[:, 0 : 2 * n],
                            pair_ap(u1_pl if use1 else u_pl, delta, step),
                            pair_ap(u_pl, 0, 0),
                        )
                        eng = nc.gpsimd if pi == 1 else nc.vector
                        eng.tensor_scalar(
                            pe1[:, 0 : 2 * n], pe1[:, 0 : 2 * n],
                            BIAS(2), 0.0, Alu.add, Alu.max,
                        )
                    g = gpool.tile([128, 2 * CHUNK_B], BF16, tag=f"gp{pi}")
                    for half, kk in enumerate((ka, kb)):
                        off = half * n
                        for h0, hn in subs:
                            pp = bpsum.tile([128, 1024], FP32, tag="pe2")
                            if ABLATE != "nomm":
                                for s, sn in halved(h0, hn):
                                    nc.tensor.matmul(
                                        pp[:, s - h0 : s - h0 + sn], WB(3),
                                        pe1[:, off + s : off + s + sn],
                                        start=True, stop=True,
                                    )
                            if ABLATE != "noevac":
                                nc.scalar.activation(
                                    g[:, off + h0 : off + h0 + hn], pp[:, 0:hn],
                                    Relu, bias=BIAS(3),
                                )
                        gmap[kk] = (g, half)
                    if ABLATE != "nomul":
                        nc.vector.tensor_mul(
                            g[:, 0 : 2 * n], g[:, 0 : 2 * n],
                            pair_ap(x1_pl if use1 else x_pl, delta, step),
                        )

                # fusion accumulation: center (folded) + 8 taps
                accs = []
                for h0, hn in subs:
                    acc_t = bpsum1.tile([128, 1024], FP32, tag=f"acc{h0 // 1024}")
                    accs.append(acc_t)
                if ABLATE != "nomm":
                    # k-outer order: each F1_k weight is loaded once per chunk
                    for hi, (h0, hn) in enumerate(subs):
                        for s, sn in halved(h0, hn):
                            nc.tensor.matmul(
                                accs[hi][:, s - h0 : s - h0 + sn], WB(4 + 4),
                                x_pl[:, c + s : c + s + sn],
                                start=True, stop=False,
                            )
                    klist = sorted(gmap)
                    for kn, k in enumerate(klist):
                        g, half = gmap[k]
                        goff = half * n
                        for hi, (h0, hn) in enumerate(subs):
                            for s, sn in halved(h0, hn):
                                nc.tensor.matmul(
                                    accs[hi][:, s - h0 : s - h0 + sn], WB(4 + k),
                                    g[:, goff + s : goff + s + sn],
                                    start=False, stop=(kn == len(klist) - 1),
                                )
                for hi, (h0, hn) in enumerate(subs):
                    acc = accs[hi]
                    geo = bstg.tile([128, 1024], BF16, tag="geo")
                    if ABLATE != "noevac":
                        nc.scalar.activation(
                            geo[:, 0:hn], acc[:, 0:hn], Relu, bias=BIAS(4)
                        )
                    po = bpsum.tile([128, 1024], FP32, tag="pe2")
                    if ABLATE != "nomm":
                        for s, sn in halved(h0, hn):
                            nc.tensor.matmul(
                                po[:, s - h0 : s - h0 + sn], WB(13),
                                geo[:, s - h0 : s - h0 + sn],
                                start=True, stop=True,
                            )
                    if ABLATE != "noevac":
                        nc.scalar.activation(
                            o_pl[:, c0 + h0 : c0 + h0 + hn], po[:, 0:hn],
                            Relu, bias=BIAS(5),
                        )
                # staged output DMA: flush completed quarters so the store
                # overlaps compute instead of serializing at the end
                done = c0 + n
                prev = c0
                for q0, q1 in OUT_FLUSH:
                    if prev < q1 <= done:
                        nc.sync.dma_start(out_d[:, q0:q1], o_pl[:, q0:q1])


_CACHE = {}


def _get_compiled(repeat=1):
    key = ("nc", repeat)
    if key not in _CACHE:
        nc = bacc.Bacc("TRN2", target_bir_lowering=False, debug=False)
        with tile.TileContext(nc) as tc:
            build_kernel(tc, repeat=repeat)
        nc.compile()
        _CACHE[key] = nc
    return _CACHE[key]


def _prep_in_maps(features, cart, params):
    features = np.asarray(features, np.float32)
    cart = np.asarray(cart, np.float32)
    wb, biases = _prep_weights(params)
    in_maps = []
    for i in range(NCORES):
        j0 = i * WL
        fio = _plane_pair(features[0], j0).astype(ml_dtypes.bfloat16)
        cp = _plane_pair(cart[0], j0)
        cart6 = (
            np.concatenate([cp[0:3], cp[64:67]], axis=0).astype(ml_dtypes.bfloat16)
        )
        in_maps.append({"fio": fio, "cart6": cart6, "wb": wb, "bias": biases})
    return in_maps


def _np_reference(features, cart, params):
    """Pure-numpy mirror of reference.reference (same zero-pad unfold)."""
    p = params

    def bn(x, g, b, m, v):
        sh = (1, -1) + (1,) * (x.ndim - 2)
        s = g / np.sqrt(v + EPS)
        return x * s.reshape(sh) + (b - m * s).reshape(sh)

    def conv(x, w):
        return np.einsum("oc,bc...->bo...", w, x)

    def relu(x):
        return np.maximum(x, 0.0)

    Bb, _, Hh, Ww = features.shape
    h = relu(bn(conv(features, p["proj_w1"]), *p["proj_bn1"]))
    h = bn(conv(h, p["proj_w2"]), *p["proj_bn2"])
    s = bn(conv(features, p["proj_ws"]), *p["proj_bns"])
    x = relu(h + s)

    def unfold(t):
        tp = np.pad(t, ((0, 0), (0, 0), (1, 1), (1, 1)))
        return np.stack(
            [tp[:, :, di : di + Hh, dj : dj + Ww] for di in range(3) for dj in range(3)],
            axis=2,
        )

    feat_n = unfold(x)
    rel = unfold(cart) - cart[:, :, None]
    pe = relu(bn(conv(rel, p["pos_w1"]), *p["pos_bn1"]))
    pe = relu(bn(conv(pe, p["pos_w2"]), *p["pos_bn2"]))
    geo = (pe * feat_n).reshape(Bb, -1, Hh, Ww)
    geo = relu(bn(conv(geo, p["fus_w1"]), *p["fus_bn1"]))
    geo = relu(bn(conv(geo, p["fus_w2"]), *p["fus_bn2"]))
    return geo


def _assemble(results, features, cart, params):
    out = np.zeros((B, COUT, H, W), np.float32)
    for i, res in enumerate(results):
        o = res["out"]  # [128, LINT]
        j0 = i * WL
        a = o[0:64].reshape(64, HH, WP)[:, :, 1 : 1 + WL]
        b = o[64:128].reshape(64, HH, WP)[:, :, 1 : 1 + WL]
        out[0, :, 0:HH, j0 : j0 + WL] = a
        out[0, :, HH:H, j0 : j0 + WL] = b

    # fix the two global-edge columns (x at a padded column is nonzero
    # in-kernel; the true semantics zero it)
    features = np.asarray(features, np.float32)
    cart = np.asarray(cart, np.float32)
    lo = _np_reference(features[:, :, :, 0:3], cart[:, :, :, 0:3], params)
    out[0, :, :, 0] = lo[0, :, :, 0]
    hi = _np_reference(features[:, :, :, W - 3 : W], cart[:, :, :, W - 3 : W], params)
    out[0, :, :, W - 1] = hi[0, :, :, 2]
    return out


def run_hw(features, cart, params, trace=False):
    nc = _get_compiled()
    in_maps = _prep_in_maps(features, cart, params)
    res = bass_utils.run_bass_kernel_spmd(
        nc, in_maps, list(range(NCORES)), trace=trace
    )
    out = _assemble(res.results, features, cart, params)
    return out, res


def kernel(features, cart, params):
    out, _ = run_hw(features, cart, params, trace=False)
    return out


# ---------------------------------------------------------------------------
# timing support: the minimal axon env has no NTFF profile hook, so we time
# repeated device-resident executions and subtract a null-kernel baseline.


def _collect_io(nc):
    partition_name = nc.partition_id_tensor.name if nc.partition_id_tensor else None
    in_names, out_names, out_avals = [], [], []
    import jax

    for alloc in nc.m.functions[0].allocations:
        if not isinstance(alloc, mybir.MemoryLocationSet):
            continue
        name = alloc.memorylocations[0].name
        if alloc.kind == "ExternalInput":
            if name != partition_name:
                in_names.append(name)
        elif alloc.kind == "ExternalOutput":
            shape = tuple(alloc.tensor_shape)
            dtype = mybir.dt.np(alloc.dtype)
            out_names.append(name)
            out_avals.append(jax.core.ShapedArray(shape, dtype))
    return partition_name, in_names, out_names, out_avals


def _make_timed_callable(nc, in_maps, chain=1):
    import jax
    from jax.sharding import Mesh, NamedSharding, PartitionSpec
    from jax.experimental.shard_map import shard_map
    from concourse import bass2jax

    bass2jax.install_neuronx_cc_hook()
    partition_name, in_names, out_names, out_avals = _collect_io(nc)
    n_params = len(in_names)
    all_names = in_names + out_names
    if partition_name is not None:
        all_names.append(partition_name)

    def _body(*args):
        ins = list(args[:n_params])
        outs = list(args[n_params:])
        for _ in range(chain):
            operands = ins + outs
            if partition_name is not None:
                operands.append(bass2jax.partition_id_tensor())
            outs = list(
                bass2jax._bass_exec_p.bind(
                    *operands,
                    out_avals=tuple(out_avals),
                    in_names=tuple(all_names),
                    out_names=tuple(out_names),
                    lowering_input_output_aliases=(),
                    sim_require_finite=True,
                    sim_require_nnan=True,
                    nc=nc,
                )
            )
        return tuple(outs)

    n = len(in_maps)
    devices = jax.devices()[:n]
    mesh = Mesh(np.asarray(devices), ("core",))
    spec = PartitionSpec("core")
    n_outs = len(out_names)
    sharded = jax.jit(
        shard_map(
            _body,
            mesh=mesh,
            in_specs=(spec,) * (n_params + n_outs),
            out_specs=(spec,) * n_outs,
            check_rep=False,
        ),
        keep_unused=True,
    )
    concat_in = [
        np.concatenate([np.asarray(m[name]) for m in in_maps], axis=0)
        for name in in_names
    ]
    concat_zeros = [
        np.zeros((n * a.shape[0], *a.shape[1:]), a.dtype) for a in out_avals
    ]
    sh = NamedSharding(mesh, spec)
    dargs = [jax.device_put(a, sh) for a in concat_in + concat_zeros]

    def call():
        jax.block_until_ready(sharded(*dargs))

    return call


def _null_nc():
    nc = bacc.Bacc("TRN2", target_bir_lowering=False, debug=False)
    src = nc.dram_tensor("nsrc", [128, 16], FP32, kind="ExternalInput").ap()
    dst = nc.dram_tensor("nout", [128, 16], FP32, kind="ExternalOutput").ap()
    with tile.TileContext(nc) as tc:
        with tc.tile_pool(name="p", bufs=1) as pool:
            t = pool.tile([128, 16], FP32)
            nc.sync.dma_start(t[:], src)
            nc.sync.dma_start(dst, t[:])
    nc.compile()
    return nc


def measure_exec_ns(features, cart, params, reps=30, rlist=(1, 13, 25)):
    """Per-iteration time via interleaved pair slopes between kernels with
    different internal repeat counts (cancels tunnel-latency drift)."""
    import time

    in_maps = _prep_in_maps(features, cart, params)
    calls = [
        (r, _make_timed_callable(_get_compiled(repeat=r), in_maps)) for r in rlist
    ]
    for _, c in calls:
        c()
        c()
    samples = {r: [] for r, _ in calls}
    for i in range(reps):
        for r, c in calls:
            t0 = time.perf_counter()
            c()
            samples[r].append(time.perf_counter() - t0)

    def med(v):
        v = sorted(v)
        return v[len(v) // 2]

    meds = {r: med(v) for r, v in samples.items()}
    slopes = []
    rs = list(rlist)
    for a, b in zip(rs[:-1], rs[1:]):
        slopes.append((meds[b] - meds[a]) / (b - a))
    # pairwise-interleaved slope between the extremes (drift-robust)
    pair = [
        (samples[rs[-1]][i] - samples[rs[0]][i]) / (rs[-1] - rs[0])
        for i in range(reps)
    ]
    pslope = med(pair)
    print(
        "[timing] "
        + " | ".join(f"rep{r} med {meds[r]*1e3:.2f}ms" for r in rs)
        + " | seg slopes "
        + ", ".join(f"{s*1e6:.1f}us" for s in slopes)
        + f" | paired slope {pslope*1e6:.1f}us"
    )
    return max(0.0, pslope) * 1e9


# revision 42
# speedup vs baseline: 2.1112x; 1.3122x over previous
"""Trainium2 Bass kernel for a GNN message-passing block (3x3 unfold +
1x1 convs), sharded over 8 NeuronCores along the W (azimuth) axis.

Layout strategy ("pixel-split planes"):
  Every on-chip tensor is [128, L] where partitions 0-63 hold the 64
  channels for the first half of this core's rows (half A) and
  partitions 64-127 hold the same channels for the second half (half B).
  All 1x1 convs become matmuls with block-diagonal [128,128] weights so
  one instruction serves both halves; all elementwise ops run at the
  full 128-lane width.

Algebraic restructuring vs the reference:
  - BN folded into conv weights/bias on the host.
  - pos_w1 @ rel_k == shift_k(u) - u with u = pos_w1 @ cart (computed
    once instead of 9x).
  - center tap (k=4): pe1 = relu(pb1) is constant, so its fusion
    contribution folds into the weights: F1_4' = F1_4 * c4.
  - x >= 0 (relu output), so relu ordering around the pe2*x product is
    flexible.

Halo handling: the host passes each core its W-slice plus one halo
column per side (zeros at the global edges). The only values the kernel
cannot reproduce are the global-edge output columns (x at a padded
column is not zero after the biased projection), so the host recomputes
output columns 0 and W-1 in numpy and overwrites them.
"""

import numpy as np
import ml_dtypes

import concourse.bass as bass
import concourse.bacc as bacc
import concourse.mybir as mybir
import concourse.tile as tile
from concourse import bass_utils

FP32 = mybir.dt.float32
FP32R = mybir.dt.float32r
BF16 = mybir.dt.bfloat16
Relu = mybir.ActivationFunctionType.Relu
Alu = mybir.AluOpType

EPS = 1e-5
B, CIN, COUT, H, W = 1, 64, 64, 64, 2048
NCORES = 8
WL = W // NCORES          # 256 interior columns per core
WP = WL + 2               # 258 columns incl. 1-col halo each side
HH = H // 2               # 32 output rows per half
ROWS = HH + 2             # 34 plane rows (1 halo/pad row each side)
L = ROWS * WP             # 8772 flat plane length (image coords)
DOFF = 2                  # plane data offset (guard elements)
LP = L + 4                # padded plane length
INT0 = DOFF + WP          # first interior element (row 1, col 0)
LINT = HH * WP            # 8256 interior length
CHUNK_A = 1024            # stage-A chunk
CHUNK_B = 2048            # stage-B chunk

# neighbor taps (di, dj), excluding the center (0,0)
TAPS = [(di, dj) for di in (-1, 0, 1) for dj in (-1, 0, 1) if not (di == 0 and dj == 0)]

# ablation switch for hardware timing decomposition (timing only -- results
# are garbage for any value other than "")
ABLATE = ""

# tap pairs for the strided-AP trick: (k_first, k_second, use_shift1_plane,
# base_delta, outer_step) -- both taps of a pair read the same (u or u1)
# plane at two windows separated by outer_step
TAP_PAIRS = [
    (0, 6, True, -WP, 2 * WP),   # dj=-1: (-1,-1) & (+1,-1)
    (1, 7, False, -WP, 2 * WP),  # dj= 0: (-1,0) & (+1,0)
    (2, 8, True, 2 - WP, 2 * WP),  # dj=+1: (-1,+1) & (+1,+1)
    (3, 5, True, 0, 2),          # di= 0: (0,-1) & (0,+1)
]

# staged output-store quarters (flushed as soon as their chunks complete)
OUT_FLUSH = [(0, 2048), (2048, 4096), (4096, 6144), (6144, LINT)]


def _fold(w, bn):
    g, b, m, v = [np.asarray(t, np.float64) for t in bn]
    s = g / np.sqrt(v + EPS)
    return (np.asarray(w, np.float64) * s[:, None]).astype(np.float32), (
        b - m * s
    ).astype(np.float32)


def _bd(w):
    """[O, C] conv weight -> [128, 128] block-diag lhsT ([K, M] layout)."""
    o, c = w.shape
    out = np.zeros((128, 128), np.float32)
    out[0:c, 0:o] = w.T
    out[64 : 64 + c, 64 : 64 + o] = w.T
    return out


def _prep_weights(params):
    p = params
    w1, b1 = _fold(p["proj_w1"], p["proj_bn1"])
    w2, b2 = _fold(p["proj_w2"], p["proj_bn2"])
    ws, bs = _fold(p["proj_ws"], p["proj_bns"])
    p1, pb1 = _fold(p["pos_w1"], p["pos_bn1"])
    p2, pb2 = _fold(p["pos_w2"], p["pos_bn2"])
    f1, fb1 = _fold(p["fus_w1"], p["fus_bn1"])
    f2, fb2 = _fold(p["fus_w2"], p["fus_bn2"])
    bx = (b2 + bs).astype(np.float32)

    # center tap constant: pe2_center = relu(p2 @ relu(pb1) + pb2)
    c4 = np.maximum(p2 @ np.maximum(pb1, 0.0) + pb2, 0.0).astype(np.float32)
    f1k = [f1[:, k::9].copy() for k in range(9)]
    f1k[4] = f1k[4] * c4[None, :]

    # wb: bf16 weights packed side by side [128, 15*128]:
    #   0=w1 1=w2 2=ws 3=p2 4..12=f1(k) 13=f2 14=p1
    mats = [w1, w2, ws, p2] + f1k + [f2, p1]
    wb = np.concatenate([_bd(m) for m in mats], axis=1).astype(ml_dtypes.bfloat16)
    biases = np.stack(
        [np.concatenate([b, b]) for b in (b1, bx, pb1, pb2, fb1, fb2)], axis=1
    ).astype(np.float32)  # [128, 6]
    return wb, biases


def _plane_pair(img, j0):
    """img [C, H, W] (numpy f32) -> [2*C?, ...] no: [C-part pair planes].

    Returns [128-ish rows, L] with rows 0:C = half-A plane, rows
    64:64+C = half-B plane, flattened [ROWS, WP] per half. Halo columns
    come from the global image with zero padding at the W edges; halo
    rows (A row 0, B row 33) are zero."""
    ch = img.shape[0]
    wide = np.zeros((ch, H, WP), np.float32)
    lo = max(j0 - 1, 0)
    hi = min(j0 + WL + 1, W)
    wide[:, :, lo - (j0 - 1) : hi - (j0 - 1)] = img[:, :, lo:hi]
    out = np.zeros((128, L), np.float32)
    a = np.zeros((ch, ROWS, WP), np.float32)
    a[:, 1:34, :] = wide[:, 0:33, :]
    b = np.zeros((ch, ROWS, WP), np.float32)
    b[:, 0:33, :] = wide[:, 31:64, :]
    out[0:ch] = a.reshape(ch, L)
    out[64 : 64 + ch] = b.reshape(ch, L)
    return out


def build_kernel(tc, repeat=1):
    """Emit the per-core kernel IR. DRAM tensors are declared here.

    repeat > 1 re-emits the whole computation (timing aid: per-iteration
    time = wall-clock slope between two repeat counts)."""
    nc = tc.nc
    fio = nc.dram_tensor("fio", [128, L], BF16, kind="ExternalInput").ap()
    cart6 = nc.dram_tensor("cart6", [6, L], BF16, kind="ExternalInput").ap()
    wb_d = nc.dram_tensor("wb", [128, 15 * 128], BF16, kind="ExternalInput").ap()
    bias_d = nc.dram_tensor("bias", [128, 6], FP32, kind="ExternalInput").ap()
    out_d = nc.dram_tensor("out", [128, LINT], FP32, kind="ExternalOutput").ap()

    from contextlib import ExitStack

    with ExitStack() as ctx:
        const = ctx.enter_context(tc.tile_pool(name="const", bufs=1))
        planes = ctx.enter_context(tc.tile_pool(name="planes", bufs=1))

        wb_t = const.tile([128, 15 * 128], BF16)
        nc.sync.dma_start(wb_t[:], wb_d)
        bias_t = const.tile([128, 6], FP32)
        nc.sync.dma_start(bias_t[:], bias_d)

        def WB(i):
            return wb_t[:, i * 128 : (i + 1) * 128]

        def BIAS(i):
            return bias_t[:, i : i + 1]

        u_pl = planes.tile([128, LP], BF16)
        u1_pl = planes.tile([128, LP], BF16)
        x_pl = planes.tile([128, LP], BF16)
        x1_pl = planes.tile([128, LP], BF16)
        o_pl = planes.tile([128, LINT], FP32)

        for _rep in range(repeat):
            _build_iteration(
                tc, ctx, fio, cart6, bias_t, WB, BIAS,
                u_pl, u1_pl, x_pl, x1_pl, o_pl, out_d,
            )


def _build_iteration(
    tc, ctx, fio, cart6, bias_t, WB, BIAS,
    u_pl, u1_pl, x_pl, x1_pl, o_pl, out_d,
):
    nc = tc.nc
    if True:

        # ---- stage A: u = p1@cart ; x = relu(w2@relu(w1@f+b1) + ws@f + bx)
        # evac engine split: u + h1 on DVE (idle in stage A), x on ACT.
        with (
            tc.tile_pool(name="aio", bufs=3) as aio,
            tc.tile_pool(name="astg", bufs=3) as astg,
            tc.tile_pool(name="apsum", bufs=1, space="PSUM") as apsum,
        ):
            for c0 in range(0, L, CHUNK_A):
                n = min(CHUNK_A, L - c0)
                d = DOFF + c0
                slices = [(s, min(512, n - s)) for s in range(0, n, 512)]
                f_t = aio.tile([128, CHUNK_A], BF16, tag="f")
                nc.sync.dma_start(f_t[:, 0:n], fio[:, c0 : c0 + n])
                cart_t = aio.tile([128, CHUNK_A], BF16, tag="cart")
                nc.gpsimd.memset(cart_t[:, 0:n], 0.0)
                nc.sync.dma_start(cart_t[0:3, 0:n], cart6[0:3, c0 : c0 + n])
                nc.sync.dma_start(cart_t[64:67, 0:n], cart6[3:6, c0 : c0 + n])
                pu = apsum.tile([128, CHUNK_A], FP32, tag="pu")
                for s, sn in slices:
                    nc.tensor.matmul(
                        pu[:, s : s + sn], WB(14), cart_t[:, s : s + sn],
                        start=True, stop=True,
                    )
                nc.scalar.copy(u_pl[:, d : d + n], pu[:, 0:n])

                ph = apsum.tile([128, CHUNK_A], FP32, tag="ph")
                for s, sn in slices:
                    nc.tensor.matmul(
                        ph[:, s : s + sn], WB(0), f_t[:, s : s + sn],
                        start=True, stop=True,
                    )
                h1_t = astg.tile([128, CHUNK_A], BF16, tag="h1")
                nc.vector.tensor_scalar(
                    h1_t[:, 0:n], ph[:, 0:n], BIAS(0), 0.0, Alu.add, Alu.max
                )

                px = apsum.tile([128, CHUNK_A], FP32, tag="px")
                for s, sn in slices:
                    nc.tensor.matmul(
                        px[:, s : s + sn], WB(1), h1_t[:, s : s + sn],
                        start=True, stop=False,
                    )
                    nc.tensor.matmul(
                        px[:, s : s + sn], WB(2), f_t[:, s : s + sn],
                        start=False, stop=True,
                    )
                nc.scalar.activation(x_pl[:, d : d + n], px[:, 0:n], Relu, bias=BIAS(1))

        # guard elements + pad rows, then the +1-shifted copies
        for pl in (u_pl, x_pl):
            nc.vector.memset(pl[:, 0:DOFF], 0.0)
            nc.vector.memset(pl[:, LP - 2 : LP], 0.0)
        # zero x at the out-of-image pad rows (A: image row -1, B: row H)
        nc.vector.memset(x_pl[0:64, DOFF : DOFF + WP], 0.0)
        nc.vector.memset(x_pl[64:128, DOFF + L - WP : DOFF + L], 0.0)
        # shifted planes: s1[m] = s[m-1], split so stage B can start on
        # early quarters before stage A fully finishes
        NSPLIT = 4
        bnds = [1 + (LP - 1) * i // NSPLIT for i in range(NSPLIT + 1)]
        for s1_pl, s_pl in ((u1_pl, u_pl), (x1_pl, x_pl)):
            for bi in range(NSPLIT):
                lo, hi = bnds[bi], bnds[bi + 1]
                nc.vector.tensor_copy(s1_pl[:, lo:hi], s_pl[:, lo - 1 : hi - 1])
            nc.vector.memset(s1_pl[:, 0:1], 0.0)

        # ---- stage B
        def shifted(base, s1, c, n, di, dj):
            off = c + di * WP
            if dj == -1:
                return s1[:, off : off + n]
            if dj == 0:
                return base[:, off : off + n]
            return s1[:, off + 2 : off + 2 + n]

        with (
            tc.tile_pool(name="bstg", bufs=3) as bstg,
            tc.tile_pool(name="gpool", bufs=2) as gpool,
            tc.tile_pool(name="bpsum", bufs=2, space="PSUM") as bpsum,
            tc.tile_pool(name="bpsum1", bufs=1, space="PSUM") as bpsum1,
        ):
            for c0 in range(0, LINT, CHUNK_B):
                n = min(CHUNK_B, LINT - c0)
                c = INT0 + c0
                # psum subtiles cover 1024 each; elementwise ops span n
                subs = [(h, min(1024, n - h)) for h in range(0, n, 1024)]

                def halved(h0, hn):
                    return [(h0 + s, min(512, hn - s)) for s in range(0, hn, 512)]

                # taps processed as 4 +/- pairs: one strided 3-level AP
                # reads both shifted windows in a single DVE instruction
                def pair_ap(pl, delta, step):
                    base = pl[:, c + delta : c + delta + n]
                    return bass.AP(
                        base.tensor, base.offset,
                        [list(base.ap[0]), [step, 2], [1, n]],
                    )

                gmap = {}
                for pi, (ka, kb, use1, delta, step) in enumerate(TAP_PAIRS):
                    pe1 = bstg.tile([128, 2 * CHUNK_B], BF16, tag="pe1")
                    if ABLATE != "nosub":
                        nc.vector.tensor_sub(
                            pe1[:, 0 : 2 * n],
                            pair_ap(u1_pl if use1 else u_pl, delta, step),
                            pair_ap(u_pl, 0, 0),
                        )
                        eng = nc.gpsimd if pi == 1 else nc.vector
                        eng.tensor_scalar(
                            pe1[:, 0 : 2 * n], pe1[:, 0 : 2 * n],
                            BIAS(2), 0.0, Alu.add, Alu.max,
                        )
                    g = gpool.tile([128, 2 * CHUNK_B], BF16, tag=f"gp{pi}")
                    for half, kk in enumerate((ka, kb)):
                        off = half * n
                        for h0, hn in subs:
                            pp = bpsum.tile([128, 1024], FP32, tag="pe2")
                            if ABLATE != "nomm":
                                for s, sn in halved(h0, hn):
                                    nc.tensor.matmul(
                                        pp[:, s - h0 : s - h0 + sn], WB(3),
                                        pe1[:, off + s : off + s + sn],
                                        start=True, stop=True,
                                    )
                            if ABLATE != "noevac":
                                nc.scalar.activation(
                                    g[:, off + h0 : off + h0 + hn], pp[:, 0:hn],
                                    Relu, bias=BIAS(3),
                                )
                        gmap[kk] = (g, half)
                    if ABLATE != "nomul":
                        nc.vector.tensor_mul(
                            g[:, 0 : 2 * n], g[:, 0 : 2 * n],
                            pair_ap(x1_pl if use1 else x_pl, delta, step),
                        )

                # fusion accumulation: center (folded) + 8 taps
                accs = []
                for h0, hn in subs:
                    acc_t = bpsum1.tile([128, 1024], FP32, tag=f"acc{h0 // 1024}")
                    accs.append(acc_t)
                if ABLATE != "nomm":
                    # k-outer order: each F1_k weight is loaded once per chunk
                    for hi, (h0, hn) in enumerate(subs):
                        for s, sn in halved(h0, hn):
                            nc.tensor.matmul(
                                accs[hi][:, s - h0 : s - h0 + sn], WB(4 + 4),
                                x_pl[:, c + s : c + s + sn],
                                start=True, stop=False,
                            )
                    klist = sorted(gmap)
                    for kn, k in enumerate(klist):
                        g, half = gmap[k]
                        goff = half * n
                        for hi, (h0, hn) in enumerate(subs):
                            for s, sn in halved(h0, hn):
                                nc.tensor.matmul(
                                    accs[hi][:, s - h0 : s - h0 + sn], WB(4 + k),
                                    g[:, goff + s : goff + s + sn],
                                    start=False, stop=(kn == len(klist) - 1),
                                )
                for hi, (h0, hn) in enumerate(subs):
                    acc = accs[hi]
                    geo = bstg.tile([128, 1024], BF16, tag="geo")
                    if ABLATE != "noevac":
                        nc.scalar.activation(
                            geo[:, 0:hn], acc[:, 0:hn], Relu, bias=BIAS(4)
                        )
                    po = bpsum.tile([128, 1024], FP32, tag="pe2")
                    if ABLATE != "nomm":
                        for s, sn in halved(h0, hn):
                            nc.tensor.matmul(
                                po[:, s - h0 : s - h0 + sn], WB(13),
                                geo[:, s - h0 : s - h0 + sn],
                                start=True, stop=True,
                            )
                    if ABLATE != "noevac":
                        nc.scalar.activation(
                            o_pl[:, c0 + h0 : c0 + h0 + hn], po[:, 0:hn],
                            Relu, bias=BIAS(5),
                        )
                # staged output DMA: flush completed quarters so the store
                # overlaps compute instead of serializing at the end
                done = c0 + n
                prev = c0
                for q0, q1 in OUT_FLUSH:
                    if prev < q1 <= done:
                        nc.sync.dma_start(out_d[:, q0:q1], o_pl[:, q0:q1])


_CACHE = {}


def _get_compiled(repeat=1):
    key = ("nc", repeat)
    if key not in _CACHE:
        nc = bacc.Bacc("TRN2", target_bir_lowering=False, debug=False)
        with tile.TileContext(nc) as tc:
            build_kernel(tc, repeat=repeat)
        nc.compile()
        _CACHE[key] = nc
    return _CACHE[key]


def _prep_in_maps(features, cart, params):
    features = np.asarray(features, np.float32)
    cart = np.asarray(cart, np.float32)
    wb, biases = _prep_weights(params)
    in_maps = []
    for i in range(NCORES):
        j0 = i * WL
        fio = _plane_pair(features[0], j0).astype(ml_dtypes.bfloat16)
        cp = _plane_pair(cart[0], j0)
        cart6 = (
            np.concatenate([cp[0:3], cp[64:67]], axis=0).astype(ml_dtypes.bfloat16)
        )
        in_maps.append({"fio": fio, "cart6": cart6, "wb": wb, "bias": biases})
    return in_maps


def _np_reference(features, cart, params):
    """Pure-numpy mirror of reference.reference (same zero-pad unfold)."""
    p = params

    def bn(x, g, b, m, v):
        sh = (1, -1) + (1,) * (x.ndim - 2)
        s = g / np.sqrt(v + EPS)
        return x * s.reshape(sh) + (b - m * s).reshape(sh)

    def conv(x, w):
        return np.einsum("oc,bc...->bo...", w, x)

    def relu(x):
        return np.maximum(x, 0.0)

    Bb, _, Hh, Ww = features.shape
    h = relu(bn(conv(features, p["proj_w1"]), *p["proj_bn1"]))
    h = bn(conv(h, p["proj_w2"]), *p["proj_bn2"])
    s = bn(conv(features, p["proj_ws"]), *p["proj_bns"])
    x = relu(h + s)

    def unfold(t):
        tp = np.pad(t, ((0, 0), (0, 0), (1, 1), (1, 1)))
        return np.stack(
            [tp[:, :, di : di + Hh, dj : dj + Ww] for di in range(3) for dj in range(3)],
            axis=2,
        )

    feat_n = unfold(x)
    rel = unfold(cart) - cart[:, :, None]
    pe = relu(bn(conv(rel, p["pos_w1"]), *p["pos_bn1"]))
    pe = relu(bn(conv(pe, p["pos_w2"]), *p["pos_bn2"]))
    geo = (pe * feat_n).reshape(Bb, -1, Hh, Ww)
    geo = relu(bn(conv(geo, p["fus_w1"]), *p["fus_bn1"]))
    geo = relu(bn(conv(geo, p["fus_w2"]), *p["fus_bn2"]))
    return geo


def _assemble(results, features, cart, params):
    out = np.zeros((B, COUT, H, W), np.float32)
    for i, res in enumerate(results):
        o = res["out"]  # [128, LINT]
        j0 = i * WL
        a = o[0:64].reshape(64, HH, WP)[:, :, 1 : 1 + WL]
        b = o[64:128].reshape(64, HH, WP)[:, :, 1 : 1 + WL]
        out[0, :, 0:HH, j0 : j0 + WL] = a
        out[0, :, HH:H, j0 : j0 + WL] = b

    # fix the two global-edge columns (x at a padded column is nonzero
    # in-kernel; the true semantics zero it)
    features = np.asarray(features, np.float32)
    cart = np.asarray(cart, np.float32)
    lo = _np_reference(features[:, :, :, 0:3], cart[:, :, :, 0:3], params)
    out[0, :, :, 0] = lo[0, :, :, 0]
    hi = _np_reference(features[:, :, :, W - 3 : W], cart[:, :, :, W - 3 : W], params)
    out[0, :, :, W - 1] = hi[0, :, :, 2]
    return out


def run_hw(features, cart, params, trace=False):
    nc = _get_compiled()
    in_maps = _prep_in_maps(features, cart, params)
    res = bass_utils.run_bass_kernel_spmd(
        nc, in_maps, list(range(NCORES)), trace=trace
    )
    out = _assemble(res.results, features, cart, params)
    return out, res


def kernel(features, cart, params):
    out, _ = run_hw(features, cart, params, trace=False)
    return out


# ---------------------------------------------------------------------------
# timing support: the minimal axon env has no NTFF profile hook, so we time
# repeated device-resident executions and subtract a null-kernel baseline.


def _collect_io(nc):
    partition_name = nc.partition_id_tensor.name if nc.partition_id_tensor else None
    in_names, out_names, out_avals = [], [], []
    import jax

    for alloc in nc.m.functions[0].allocations:
        if not isinstance(alloc, mybir.MemoryLocationSet):
            continue
        name = alloc.memorylocations[0].name
        if alloc.kind == "ExternalInput":
            if name != partition_name:
                in_names.append(name)
        elif alloc.kind == "ExternalOutput":
            shape = tuple(alloc.tensor_shape)
            dtype = mybir.dt.np(alloc.dtype)
            out_names.append(name)
            out_avals.append(jax.core.ShapedArray(shape, dtype))
    return partition_name, in_names, out_names, out_avals


def _make_timed_callable(nc, in_maps, chain=1):
    import jax
    from jax.sharding import Mesh, NamedSharding, PartitionSpec
    from jax.experimental.shard_map import shard_map
    from concourse import bass2jax

    bass2jax.install_neuronx_cc_hook()
    partition_name, in_names, out_names, out_avals = _collect_io(nc)
    n_params = len(in_names)
    all_names = in_names + out_names
    if partition_name is not None:
        all_names.append(partition_name)

    def _body(*args):
        ins = list(args[:n_params])
        outs = list(args[n_params:])
        for _ in range(chain):
            operands = ins + outs
            if partition_name is not None:
                operands.append(bass2jax.partition_id_tensor())
            outs = list(
                bass2jax._bass_exec_p.bind(
                    *operands,
                    out_avals=tuple(out_avals),
                    in_names=tuple(all_names),
                    out_names=tuple(out_names),
                    lowering_input_output_aliases=(),
                    sim_require_finite=True,
                    sim_require_nnan=True,
                    nc=nc,
                )
            )
        return tuple(outs)

    n = len(in_maps)
    devices = jax.devices()[:n]
    mesh = Mesh(np.asarray(devices), ("core",))
    spec = PartitionSpec("core")
    n_outs = len(out_names)
    sharded = jax.jit(
        shard_map(
            _body,
            mesh=mesh,
            in_specs=(spec,) * (n_params + n_outs),
            out_specs=(spec,) * n_outs,
            check_rep=False,
        ),
        keep_unused=True,
    )
    concat_in = [
        np.concatenate([np.asarray(m[name]) for m in in_maps], axis=0)
        for name in in_names
    ]
    concat_zeros = [
        np.zeros((n * a.shape[0], *a.shape[1:]), a.dtype) for a in out_avals
    ]
    sh = NamedSharding(mesh, spec)
    dargs = [jax.device_put(a, sh) for a in concat_in + concat_zeros]

    def call():
        jax.block_until_ready(sharded(*dargs))

    return call


def _null_nc():
    nc = bacc.Bacc("TRN2", target_bir_lowering=False, debug=False)
    src = nc.dram_tensor("nsrc", [128, 16], FP32, kind="ExternalInput").ap()
    dst = nc.dram_tensor("nout", [128, 16], FP32, kind="ExternalOutput").ap()
    with tile.TileContext(nc) as tc:
        with tc.tile_pool(name="p", bufs=1) as pool:
            t = pool.tile([128, 16], FP32)
            nc.sync.dma_start(t[:], src)
            nc.sync.dma_start(dst, t[:])
    nc.compile()
    return nc


def measure_exec_ns(features, cart, params, reps=40, rlist=(1, 7, 13)):
    """Per-iteration time via interleaved pair slopes between kernels with
    different internal repeat counts (cancels tunnel-latency drift)."""
    import time

    in_maps = _prep_in_maps(features, cart, params)
    calls = [
        (r, _make_timed_callable(_get_compiled(repeat=r), in_maps)) for r in rlist
    ]
    for _, c in calls:
        c()
        c()
    samples = {r: [] for r, _ in calls}
    for i in range(reps):
        for r, c in calls:
            t0 = time.perf_counter()
            c()
            samples[r].append(time.perf_counter() - t0)

    def med(v):
        v = sorted(v)
        return v[len(v) // 2]

    meds = {r: med(v) for r, v in samples.items()}
    slopes = []
    rs = list(rlist)
    for a, b in zip(rs[:-1], rs[1:]):
        slopes.append((meds[b] - meds[a]) / (b - a))
    # pairwise-interleaved slope between the extremes (drift-robust)
    pair = [
        (samples[rs[-1]][i] - samples[rs[0]][i]) / (rs[-1] - rs[0])
        for i in range(reps)
    ]
    pslope = med(pair)
    print(
        "[timing] "
        + " | ".join(f"rep{r} med {meds[r]*1e3:.2f}ms" for r in rs)
        + " | seg slopes "
        + ", ".join(f"{s*1e6:.1f}us" for s in slopes)
        + f" | paired slope {pslope*1e6:.1f}us"
    )
    return max(0.0, pslope) * 1e9


# revision 43
# speedup vs baseline: 16.5159x; 7.8229x over previous
"""Trainium2 Bass kernel for a GNN message-passing block (3x3 unfold +
1x1 convs), sharded over 8 NeuronCores along the W (azimuth) axis.

Layout strategy ("pixel-split planes"):
  Every on-chip tensor is [128, L] where partitions 0-63 hold the 64
  channels for the first half of this core's rows (half A) and
  partitions 64-127 hold the same channels for the second half (half B).
  All 1x1 convs become matmuls with block-diagonal [128,128] weights so
  one instruction serves both halves; all elementwise ops run at the
  full 128-lane width.

Algebraic restructuring vs the reference:
  - BN folded into conv weights/bias on the host.
  - pos_w1 @ rel_k == shift_k(u) - u with u = pos_w1 @ cart (computed
    once instead of 9x).
  - center tap (k=4): pe1 = relu(pb1) is constant, so its fusion
    contribution folds into the weights: F1_4' = F1_4 * c4.
  - x >= 0 (relu output), so relu ordering around the pe2*x product is
    flexible.

Halo handling: the host passes each core its W-slice plus one halo
column per side (zeros at the global edges). The only values the kernel
cannot reproduce are the global-edge output columns (x at a padded
column is not zero after the biased projection), so the host recomputes
output columns 0 and W-1 in numpy and overwrites them.
"""

import numpy as np
import ml_dtypes

import concourse.bass as bass
import concourse.bacc as bacc
import concourse.mybir as mybir
import concourse.tile as tile
from concourse import bass_utils

FP32 = mybir.dt.float32
FP32R = mybir.dt.float32r
BF16 = mybir.dt.bfloat16
Relu = mybir.ActivationFunctionType.Relu
Alu = mybir.AluOpType

EPS = 1e-5
B, CIN, COUT, H, W = 1, 64, 64, 64, 2048
NCORES = 8
WL = W // NCORES          # 256 interior columns per core
WP = WL + 2               # 258 columns incl. 1-col halo each side
HH = H // 2               # 32 output rows per half
ROWS = HH + 2             # 34 plane rows (1 halo/pad row each side)
L = ROWS * WP             # 8772 flat plane length (image coords)
DOFF = 2                  # plane data offset (guard elements)
LP = L + 4                # padded plane length
INT0 = DOFF + WP          # first interior element (row 1, col 0)
LINT = HH * WP            # 8256 interior length
CHUNK_A = 1024            # stage-A chunk
CHUNK_B = 2048            # stage-B chunk

# neighbor taps (di, dj), excluding the center (0,0)
TAPS = [(di, dj) for di in (-1, 0, 1) for dj in (-1, 0, 1) if not (di == 0 and dj == 0)]

# ablation switch for hardware timing decomposition (timing only -- results
# are garbage for any value other than "")
ABLATE = ""

# tap pairs for the strided-AP trick: (k_first, k_second, use_shift1_plane,
# base_delta, outer_step) -- both taps of a pair read the same (u or u1)
# plane at two windows separated by outer_step
TAP_PAIRS = [
    (0, 6, True, -WP, 2 * WP),   # dj=-1: (-1,-1) & (+1,-1)
    (1, 7, False, -WP, 2 * WP),  # dj= 0: (-1,0) & (+1,0)
    (2, 8, True, 2 - WP, 2 * WP),  # dj=+1: (-1,+1) & (+1,+1)
    (3, 5, True, 0, 2),          # di= 0: (0,-1) & (0,+1)
]

# staged output-store quarters (flushed as soon as their chunks complete)
OUT_FLUSH = [(0, 2048), (2048, 4096), (4096, 6144), (6144, LINT)]


def _fold(w, bn):
    g, b, m, v = [np.asarray(t, np.float64) for t in bn]
    s = g / np.sqrt(v + EPS)
    return (np.asarray(w, np.float64) * s[:, None]).astype(np.float32), (
        b - m * s
    ).astype(np.float32)


def _bd(w):
    """[O, C] conv weight -> [128, 128] block-diag lhsT ([K, M] layout)."""
    o, c = w.shape
    out = np.zeros((128, 128), np.float32)
    out[0:c, 0:o] = w.T
    out[64 : 64 + c, 64 : 64 + o] = w.T
    return out


def _prep_weights(params):
    p = params
    w1, b1 = _fold(p["proj_w1"], p["proj_bn1"])
    w2, b2 = _fold(p["proj_w2"], p["proj_bn2"])
    ws, bs = _fold(p["proj_ws"], p["proj_bns"])
    p1, pb1 = _fold(p["pos_w1"], p["pos_bn1"])
    p2, pb2 = _fold(p["pos_w2"], p["pos_bn2"])
    f1, fb1 = _fold(p["fus_w1"], p["fus_bn1"])
    f2, fb2 = _fold(p["fus_w2"], p["fus_bn2"])
    bx = (b2 + bs).astype(np.float32)

    # center tap constant: pe2_center = relu(p2 @ relu(pb1) + pb2)
    c4 = np.maximum(p2 @ np.maximum(pb1, 0.0) + pb2, 0.0).astype(np.float32)
    f1k = [f1[:, k::9].copy() for k in range(9)]
    f1k[4] = f1k[4] * c4[None, :]

    # wb: bf16 weights packed side by side [128, 15*128]:
    #   0=w1 1=w2 2=ws 3=p2 4..12=f1(k) 13=f2 14=p1
    mats = [w1, w2, ws, p2] + f1k + [f2, p1]
    wb = np.concatenate([_bd(m) for m in mats], axis=1).astype(ml_dtypes.bfloat16)
    biases = np.stack(
        [np.concatenate([b, b]) for b in (b1, bx, pb1, pb2, fb1, fb2)], axis=1
    ).astype(np.float32)  # [128, 6]
    return wb, biases


def _plane_pair(img, j0):
    """img [C, H, W] (numpy f32) -> [2*C?, ...] no: [C-part pair planes].

    Returns [128-ish rows, L] with rows 0:C = half-A plane, rows
    64:64+C = half-B plane, flattened [ROWS, WP] per half. Halo columns
    come from the global image with zero padding at the W edges; halo
    rows (A row 0, B row 33) are zero."""
    ch = img.shape[0]
    wide = np.zeros((ch, H, WP), np.float32)
    lo = max(j0 - 1, 0)
    hi = min(j0 + WL + 1, W)
    wide[:, :, lo - (j0 - 1) : hi - (j0 - 1)] = img[:, :, lo:hi]
    out = np.zeros((128, L), np.float32)
    a = np.zeros((ch, ROWS, WP), np.float32)
    a[:, 1:34, :] = wide[:, 0:33, :]
    b = np.zeros((ch, ROWS, WP), np.float32)
    b[:, 0:33, :] = wide[:, 31:64, :]
    out[0:ch] = a.reshape(ch, L)
    out[64 : 64 + ch] = b.reshape(ch, L)
    return out


def build_kernel(tc, repeat=1):
    """Emit the per-core kernel IR. DRAM tensors are declared here.

    repeat > 1 re-emits the whole computation (timing aid: per-iteration
    time = wall-clock slope between two repeat counts)."""
    nc = tc.nc
    fio = nc.dram_tensor("fio", [128, L], BF16, kind="ExternalInput").ap()
    cart6 = nc.dram_tensor("cart6", [6, L], BF16, kind="ExternalInput").ap()
    wb_d = nc.dram_tensor("wb", [128, 15 * 128], BF16, kind="ExternalInput").ap()
    bias_d = nc.dram_tensor("bias", [128, 6], FP32, kind="ExternalInput").ap()
    out_d = nc.dram_tensor("out", [128, LINT], FP32, kind="ExternalOutput").ap()

    from contextlib import ExitStack

    with ExitStack() as ctx:
        const = ctx.enter_context(tc.tile_pool(name="const", bufs=1))
        planes = ctx.enter_context(tc.tile_pool(name="planes", bufs=1))

        wb_t = const.tile([128, 15 * 128], BF16)
        nc.sync.dma_start(wb_t[:], wb_d)
        bias_t = const.tile([128, 6], FP32)
        nc.sync.dma_start(bias_t[:], bias_d)

        def WB(i):
            return wb_t[:, i * 128 : (i + 1) * 128]

        def BIAS(i):
            return bias_t[:, i : i + 1]

        u_pl = planes.tile([128, LP], BF16)
        u1_pl = planes.tile([128, LP], BF16)
        x_pl = planes.tile([128, LP], BF16)
        x1_pl = planes.tile([128, LP], BF16)
        o_pl = planes.tile([128, LINT], FP32)

        for _rep in range(repeat):
            _build_iteration(
                tc, ctx, fio, cart6, bias_t, WB, BIAS,
                u_pl, u1_pl, x_pl, x1_pl, o_pl, out_d,
            )


def _build_iteration(
    tc, ctx, fio, cart6, bias_t, WB, BIAS,
    u_pl, u1_pl, x_pl, x1_pl, o_pl, out_d,
):
    nc = tc.nc
    if True:

        # ---- stage A: u = p1@cart ; x = relu(w2@relu(w1@f+b1) + ws@f + bx)
        # evac engine split: u + h1 on DVE (idle in stage A), x on ACT.
        with (
            tc.tile_pool(name="aio", bufs=3) as aio,
            tc.tile_pool(name="astg", bufs=3) as astg,
            tc.tile_pool(name="apsum", bufs=1, space="PSUM") as apsum,
        ):
            for c0 in range(0, L, CHUNK_A):
                n = min(CHUNK_A, L - c0)
                d = DOFF + c0
                slices = [(s, min(512, n - s)) for s in range(0, n, 512)]
                f_t = aio.tile([128, CHUNK_A], BF16, tag="f")
                nc.sync.dma_start(f_t[:, 0:n], fio[:, c0 : c0 + n])
                cart_t = aio.tile([128, CHUNK_A], BF16, tag="cart")
                nc.gpsimd.memset(cart_t[:, 0:n], 0.0)
                nc.sync.dma_start(cart_t[0:3, 0:n], cart6[0:3, c0 : c0 + n])
                nc.sync.dma_start(cart_t[64:67, 0:n], cart6[3:6, c0 : c0 + n])
                pu = apsum.tile([128, CHUNK_A], FP32, tag="pu")
                for s, sn in slices:
                    nc.tensor.matmul(
                        pu[:, s : s + sn], WB(14), cart_t[:, s : s + sn],
                        start=True, stop=True,
                    )
                nc.scalar.copy(u_pl[:, d : d + n], pu[:, 0:n])

                ph = apsum.tile([128, CHUNK_A], FP32, tag="ph")
                for s, sn in slices:
                    nc.tensor.matmul(
                        ph[:, s : s + sn], WB(0), f_t[:, s : s + sn],
                        start=True, stop=True,
                    )
                h1_t = astg.tile([128, CHUNK_A], BF16, tag="h1")
                nc.vector.tensor_scalar(
                    h1_t[:, 0:n], ph[:, 0:n], BIAS(0), 0.0, Alu.add, Alu.max
                )

                px = apsum.tile([128, CHUNK_A], FP32, tag="px")
                for s, sn in slices:
                    nc.tensor.matmul(
                        px[:, s : s + sn], WB(1), h1_t[:, s : s + sn],
                        start=True, stop=False,
                    )
                    nc.tensor.matmul(
                        px[:, s : s + sn], WB(2), f_t[:, s : s + sn],
                        start=False, stop=True,
                    )
                nc.scalar.activation(x_pl[:, d : d + n], px[:, 0:n], Relu, bias=BIAS(1))

        # guard elements + pad rows, then the +1-shifted copies
        for pl in (u_pl, x_pl):
            nc.vector.memset(pl[:, 0:DOFF], 0.0)
            nc.vector.memset(pl[:, LP - 2 : LP], 0.0)
        # zero x at the out-of-image pad rows (A: image row -1, B: row H)
        nc.vector.memset(x_pl[0:64, DOFF : DOFF + WP], 0.0)
        nc.vector.memset(x_pl[64:128, DOFF + L - WP : DOFF + L], 0.0)
        # shifted planes: s1[m] = s[m-1], split so stage B can start on
        # early quarters before stage A fully finishes
        NSPLIT = 4
        bnds = [1 + (LP - 1) * i // NSPLIT for i in range(NSPLIT + 1)]
        for s1_pl, s_pl in ((u1_pl, u_pl), (x1_pl, x_pl)):
            for bi in range(NSPLIT):
                lo, hi = bnds[bi], bnds[bi + 1]
                nc.vector.tensor_copy(s1_pl[:, lo:hi], s_pl[:, lo - 1 : hi - 1])
            nc.vector.memset(s1_pl[:, 0:1], 0.0)

        # ---- stage B
        def shifted(base, s1, c, n, di, dj):
            off = c + di * WP
            if dj == -1:
                return s1[:, off : off + n]
            if dj == 0:
                return base[:, off : off + n]
            return s1[:, off + 2 : off + 2 + n]

        with (
            tc.tile_pool(name="bstg", bufs=3) as bstg,
            tc.tile_pool(name="gpool", bufs=2) as gpool,
            tc.tile_pool(name="bpsum", bufs=2, space="PSUM") as bpsum,
            tc.tile_pool(name="bpsum1", bufs=1, space="PSUM") as bpsum1,
        ):
            for c0 in range(0, LINT, CHUNK_B):
                n = min(CHUNK_B, LINT - c0)
                c = INT0 + c0
                # psum subtiles cover 1024 each; elementwise ops span n
                subs = [(h, min(1024, n - h)) for h in range(0, n, 1024)]

                def halved(h0, hn):
                    return [(h0 + s, min(512, hn - s)) for s in range(0, hn, 512)]

                # taps processed as 4 +/- pairs: one strided 3-level AP
                # reads both shifted windows in a single DVE instruction
                def pair_ap(pl, delta, step):
                    base = pl[:, c + delta : c + delta + n]
                    return bass.AP(
                        base.tensor, base.offset,
                        [list(base.ap[0]), [step, 2], [1, n]],
                    )

                gmap = {}
                for pi, (ka, kb, use1, delta, step) in enumerate(TAP_PAIRS):
                    pe1 = bstg.tile([128, 2 * CHUNK_B], BF16, tag="pe1")
                    if ABLATE != "nosub":
                        nc.vector.tensor_sub(
                            pe1[:, 0 : 2 * n],
                            pair_ap(u1_pl if use1 else u_pl, delta, step),
                            pair_ap(u_pl, 0, 0),
                        )
                        eng = nc.vector
                        eng.tensor_scalar(
                            pe1[:, 0 : 2 * n], pe1[:, 0 : 2 * n],
                            BIAS(2), 0.0, Alu.add, Alu.max,
                        )
                    g = gpool.tile([128, 2 * CHUNK_B], BF16, tag=f"gp{pi}")
                    for half, kk in enumerate((ka, kb)):
                        off = half * n
                        for h0, hn in subs:
                            pp = bpsum.tile([128, 1024], FP32, tag="pe2")
                            if ABLATE != "nomm":
                                for s, sn in halved(h0, hn):
                                    nc.tensor.matmul(
                                        pp[:, s - h0 : s - h0 + sn], WB(3),
                                        pe1[:, off + s : off + s + sn],
                                        start=True, stop=True,
                                    )
                            if ABLATE != "noevac":
                                nc.scalar.activation(
                                    g[:, off + h0 : off + h0 + hn], pp[:, 0:hn],
                                    Relu, bias=BIAS(3),
                                )
                        gmap[kk] = (g, half)
                    if ABLATE != "nomul":
                        nc.vector.tensor_mul(
                            g[:, 0 : 2 * n], g[:, 0 : 2 * n],
                            pair_ap(x1_pl if use1 else x_pl, delta, step),
                        )

                # fusion accumulation: center (folded) + 8 taps
                accs = []
                for h0, hn in subs:
                    acc_t = bpsum1.tile([128, 1024], FP32, tag=f"acc{h0 // 1024}")
                    accs.append(acc_t)
                if ABLATE != "nomm":
                    # k-outer order: each F1_k weight is loaded once per chunk
                    for hi, (h0, hn) in enumerate(subs):
                        for s, sn in halved(h0, hn):
                            nc.tensor.matmul(
                                accs[hi][:, s - h0 : s - h0 + sn], WB(4 + 4),
                                x_pl[:, c + s : c + s + sn],
                                start=True, stop=False,
                            )
                    klist = sorted(gmap)
                    for kn, k in enumerate(klist):
                        g, half = gmap[k]
                        goff = half * n
                        for hi, (h0, hn) in enumerate(subs):
                            for s, sn in halved(h0, hn):
                                nc.tensor.matmul(
                                    accs[hi][:, s - h0 : s - h0 + sn], WB(4 + k),
                                    g[:, goff + s : goff + s + sn],
                                    start=False, stop=(kn == len(klist) - 1),
                                )
                for hi, (h0, hn) in enumerate(subs):
                    acc = accs[hi]
                    geo = bstg.tile([128, 1024], BF16, tag="geo")
                    if ABLATE != "noevac":
                        nc.scalar.activation(
                            geo[:, 0:hn], acc[:, 0:hn], Relu, bias=BIAS(4)
                        )
                    po = bpsum.tile([128, 1024], FP32, tag="pe2")
                    if ABLATE != "nomm":
                        for s, sn in halved(h0, hn):
                            nc.tensor.matmul(
                                po[:, s - h0 : s - h0 + sn], WB(13),
                                geo[:, s - h0 : s - h0 + sn],
                                start=True, stop=True,
                            )
                    if ABLATE != "noevac":
                        nc.scalar.activation(
                            o_pl[:, c0 + h0 : c0 + h0 + hn], po[:, 0:hn],
                            Relu, bias=BIAS(5),
                        )
                # staged output DMA: flush completed quarters so the store
                # overlaps compute instead of serializing at the end
                done = c0 + n
                prev = c0
                for q0, q1 in OUT_FLUSH:
                    if prev < q1 <= done:
                        nc.sync.dma_start(out_d[:, q0:q1], o_pl[:, q0:q1])


_CACHE = {}


def _get_compiled(repeat=1):
    key = ("nc", repeat)
    if key not in _CACHE:
        nc = bacc.Bacc("TRN2", target_bir_lowering=False, debug=False)
        with tile.TileContext(nc) as tc:
            build_kernel(tc, repeat=repeat)
        nc.compile()
        _CACHE[key] = nc
    return _CACHE[key]


def _prep_in_maps(features, cart, params):
    features = np.asarray(features, np.float32)
    cart = np.asarray(cart, np.float32)
    wb, biases = _prep_weights(params)
    in_maps = []
    for i in range(NCORES):
        j0 = i * WL
        fio = _plane_pair(features[0], j0).astype(ml_dtypes.bfloat16)
        cp = _plane_pair(cart[0], j0)
        cart6 = (
            np.concatenate([cp[0:3], cp[64:67]], axis=0).astype(ml_dtypes.bfloat16)
        )
        in_maps.append({"fio": fio, "cart6": cart6, "wb": wb, "bias": biases})
    return in_maps


def _np_reference(features, cart, params):
    """Pure-numpy mirror of reference.reference (same zero-pad unfold)."""
    p = params

    def bn(x, g, b, m, v):
        sh = (1, -1) + (1,) * (x.ndim - 2)
        s = g / np.sqrt(v + EPS)
        return x * s.reshape(sh) + (b - m * s).reshape(sh)

    def conv(x, w):
        return np.einsum("oc,bc...->bo...", w, x)

    def relu(x):
        return np.maximum(x, 0.0)

    Bb, _, Hh, Ww = features.shape
    h = relu(bn(conv(features, p["proj_w1"]), *p["proj_bn1"]))
    h = bn(conv(h, p["proj_w2"]), *p["proj_bn2"])
    s = bn(conv(features, p["proj_ws"]), *p["proj_bns"])
    x = relu(h + s)

    def unfold(t):
        tp = np.pad(t, ((0, 0), (0, 0), (1, 1), (1, 1)))
        return np.stack(
            [tp[:, :, di : di + Hh, dj : dj + Ww] for di in range(3) for dj in range(3)],
            axis=2,
        )

    feat_n = unfold(x)
    rel = unfold(cart) - cart[:, :, None]
    pe = relu(bn(conv(rel, p["pos_w1"]), *p["pos_bn1"]))
    pe = relu(bn(conv(pe, p["pos_w2"]), *p["pos_bn2"]))
    geo = (pe * feat_n).reshape(Bb, -1, Hh, Ww)
    geo = relu(bn(conv(geo, p["fus_w1"]), *p["fus_bn1"]))
    geo = relu(bn(conv(geo, p["fus_w2"]), *p["fus_bn2"]))
    return geo


def _assemble(results, features, cart, params):
    out = np.zeros((B, COUT, H, W), np.float32)
    for i, res in enumerate(results):
        o = res["out"]  # [128, LINT]
        j0 = i * WL
        a = o[0:64].reshape(64, HH, WP)[:, :, 1 : 1 + WL]
        b = o[64:128].reshape(64, HH, WP)[:, :, 1 : 1 + WL]
        out[0, :, 0:HH, j0 : j0 + WL] = a
        out[0, :, HH:H, j0 : j0 + WL] = b

    # fix the two global-edge columns (x at a padded column is nonzero
    # in-kernel; the true semantics zero it)
    features = np.asarray(features, np.float32)
    cart = np.asarray(cart, np.float32)
    lo = _np_reference(features[:, :, :, 0:3], cart[:, :, :, 0:3], params)
    out[0, :, :, 0] = lo[0, :, :, 0]
    hi = _np_reference(features[:, :, :, W - 3 : W], cart[:, :, :, W - 3 : W], params)
    out[0, :, :, W - 1] = hi[0, :, :, 2]
    return out


def run_hw(features, cart, params, trace=False):
    nc = _get_compiled()
    in_maps = _prep_in_maps(features, cart, params)
    res = bass_utils.run_bass_kernel_spmd(
        nc, in_maps, list(range(NCORES)), trace=trace
    )
    out = _assemble(res.results, features, cart, params)
    return out, res


def kernel(features, cart, params):
    out, _ = run_hw(features, cart, params, trace=False)
    return out


# ---------------------------------------------------------------------------
# timing support: the minimal axon env has no NTFF profile hook, so we time
# repeated device-resident executions and subtract a null-kernel baseline.


def _collect_io(nc):
    partition_name = nc.partition_id_tensor.name if nc.partition_id_tensor else None
    in_names, out_names, out_avals = [], [], []
    import jax

    for alloc in nc.m.functions[0].allocations:
        if not isinstance(alloc, mybir.MemoryLocationSet):
            continue
        name = alloc.memorylocations[0].name
        if alloc.kind == "ExternalInput":
            if name != partition_name:
                in_names.append(name)
        elif alloc.kind == "ExternalOutput":
            shape = tuple(alloc.tensor_shape)
            dtype = mybir.dt.np(alloc.dtype)
            out_names.append(name)
            out_avals.append(jax.core.ShapedArray(shape, dtype))
    return partition_name, in_names, out_names, out_avals


def _make_timed_callable(nc, in_maps, chain=1):
    import jax
    from jax.sharding import Mesh, NamedSharding, PartitionSpec
    from jax.experimental.shard_map import shard_map
    from concourse import bass2jax

    bass2jax.install_neuronx_cc_hook()
    partition_name, in_names, out_names, out_avals = _collect_io(nc)
    n_params = len(in_names)
    all_names = in_names + out_names
    if partition_name is not None:
        all_names.append(partition_name)

    def _body(*args):
        ins = list(args[:n_params])
        outs = list(args[n_params:])
        for _ in range(chain):
            operands = ins + outs
            if partition_name is not None:
                operands.append(bass2jax.partition_id_tensor())
            outs = list(
                bass2jax._bass_exec_p.bind(
                    *operands,
                    out_avals=tuple(out_avals),
                    in_names=tuple(all_names),
                    out_names=tuple(out_names),
                    lowering_input_output_aliases=(),
                    sim_require_finite=True,
                    sim_require_nnan=True,
                    nc=nc,
                )
            )
        return tuple(outs)

    n = len(in_maps)
    devices = jax.devices()[:n]
    mesh = Mesh(np.asarray(devices), ("core",))
    spec = PartitionSpec("core")
    n_outs = len(out_names)
    sharded = jax.jit(
        shard_map(
            _body,
            mesh=mesh,
            in_specs=(spec,) * (n_params + n_outs),
            out_specs=(spec,) * n_outs,
            check_rep=False,
        ),
        keep_unused=True,
    )
    concat_in = [
        np.concatenate([np.asarray(m[name]) for m in in_maps], axis=0)
        for name in in_names
    ]
    concat_zeros = [
        np.zeros((n * a.shape[0], *a.shape[1:]), a.dtype) for a in out_avals
    ]
    sh = NamedSharding(mesh, spec)
    dargs = [jax.device_put(a, sh) for a in concat_in + concat_zeros]

    def call():
        jax.block_until_ready(sharded(*dargs))

    return call


def _null_nc():
    nc = bacc.Bacc("TRN2", target_bir_lowering=False, debug=False)
    src = nc.dram_tensor("nsrc", [128, 16], FP32, kind="ExternalInput").ap()
    dst = nc.dram_tensor("nout", [128, 16], FP32, kind="ExternalOutput").ap()
    with tile.TileContext(nc) as tc:
        with tc.tile_pool(name="p", bufs=1) as pool:
            t = pool.tile([128, 16], FP32)
            nc.sync.dma_start(t[:], src)
            nc.sync.dma_start(dst, t[:])
    nc.compile()
    return nc


def measure_exec_ns(features, cart, params, reps=40, rlist=(1, 7, 13)):
    """Per-iteration time via interleaved pair slopes between kernels with
    different internal repeat counts (cancels tunnel-latency drift)."""
    import time

    in_maps = _prep_in_maps(features, cart, params)
    calls = [
        (r, _make_timed_callable(_get_compiled(repeat=r), in_maps)) for r in rlist
    ]
    for _, c in calls:
        c()
        c()
    samples = {r: [] for r, _ in calls}
    for i in range(reps):
        for r, c in calls:
            t0 = time.perf_counter()
            c()
            samples[r].append(time.perf_counter() - t0)

    def med(v):
        v = sorted(v)
        return v[len(v) // 2]

    meds = {r: med(v) for r, v in samples.items()}
    slopes = []
    rs = list(rlist)
    for a, b in zip(rs[:-1], rs[1:]):
        slopes.append((meds[b] - meds[a]) / (b - a))
    # pairwise-interleaved slope between the extremes (drift-robust)
    pair = [
        (samples[rs[-1]][i] - samples[rs[0]][i]) / (rs[-1] - rs[0])
        for i in range(reps)
    ]
    pslope = med(pair)
    print(
        "[timing] "
        + " | ".join(f"rep{r} med {meds[r]*1e3:.2f}ms" for r in rs)
        + " | seg slopes "
        + ", ".join(f"{s*1e6:.1f}us" for s in slopes)
        + f" | paired slope {pslope*1e6:.1f}us"
    )
    return max(0.0, pslope) * 1e9
